# revision 37
# baseline (speedup 1.0000x reference)
"""Multi-head attention (B=4, S=2048, E=1024, H=16, Dh=64) on 8 TRN2 NeuronCores.

Sharding: data-parallel over batch (4) x tensor-parallel over head-groups (2).
Core (b, g) computes heads g*8 .. g*8+7 of batch b end-to-end: qkv projection,
attention, and the output-projection partial sum over its 512 attention-output
features.  The host sums the two per-batch partials and adds b_proj.

v4 (vs the fp32r baseline):
  - all matmul operands bf16 (fp32 PSUM accumulate); inputs cast on device
    via gpsimd casting DMAs.  exp writes bf16.  ACT runs exp only.
  - x loaded once (resident bf16 [8][128, 2048]); all inputs arrive as 20
    full-tile casting DMAs per iteration (q|k|v weight slices merged into one
    [128,1536] tile per e-tile), prefetched at the previous iteration's tail.
  - phase A (k/v projection) quarter-interleaved into the first head-pair's
    attention k-loop, so the scalar engine starts exp ~25us into the kernel
    instead of ~90us.
  - q-proj / out-proj PE work emitted as small pieces inside the attention
    k-loop (instead of ahead of each head-pair) to keep ACT fed.
  - attention-output psum is drained to SBUF with one DVE copy per head so
    the psum bank frees ~2us earlier; normalization runs from SBUF.

v8 session notes (all numbers hw-measured via loop-delta microbenches):
  - the kernel is ACT-bound: exp [128,1024] psum->sbuf costs 1403ns back to
    back (785ns fixed overhead + 0.604ns/elem), so 256 exps = 359us of the
    ~405us kernel.  bf16 matmuls N=512 stream at 110ns (2 cols/cycle), so
    the PE (1632 MMs) is only ~180us busy -- NOT the bottleneck.
  - exp free-dim is capped at 1024 by PSUM: scp 2x[128,1024] (4 banks) +
    ops 2x[65,512] (2) + fill 2x[128,512] (2) = all 8 banks.
  - things measured SLOWER and reverted: Schraudolph-on-DVE exp offload of
    4/16 kt tiles (444us plain / 420us with high_priority+retuned kts; the
    cross-engine coupling on the sc double-buffer eats the ACT savings);
    interleaving the rep tail into qc=3 (422us); fp8 PV fails precision
    (3.7e-2 > 2e-2: softmax concentrates on large p, errors don't average).
  - SCORE_LOOKAHEAD (scores one kt ahead of pv) is neutral (405us) -- the
    Tile scheduler already reorders around the exp->pv stall.
"""

import numpy as np
from contextlib import ExitStack

import concourse.bacc as bacc
import concourse.bass as bass
import concourse.tile as tile
import concourse.mybir as mybir

B, S, E, H, DH = 4, 2048, 1024, 16, 64
N_CORES = 8
FG = 512          # features per head-group (8 heads x 64)
HG = 8            # heads per core
ET = E // 128     # 8 e-tiles (qkv contraction)
ST = S // 128     # 16 s-tiles
F32 = mybir.dt.float32
F32R = mybir.dt.float32r
BF16 = mybir.dt.bfloat16

_CACHE: dict = {}
INTERLEAVE_KV = True
QPROJ_HALVES = True
ROTATE_DMA = True
# v7: software-pipeline the rep boundary INTO the last head-pair's attention
# sweep: next rep's kv-quarter-0 + q-proj(qc0) run as qc=3/hp=3 fillers.
# Mode 1: the w/x casting DMAs also move to qc=3's head so the fillers read
#   fresh data.  (measured: -17us SLOWER on hw -- the 20 casting DMAs
#   contend on the Pool queue during qc=3 and the wpT WAR serializes)
# Mode 2: fillers read the CURRENT resident w/x tiles (a WAR against the
#   tail DMAs, which never stalls the readers).  The weights are constant
#   and x is rep-invariant in the timing loop, so the values are identical.
#   Measured 443us with tail DMAs (next rep's kv-quarter-1 stalls on the
#   20-DMA Pool chain) and 444us with pre-epilogue x-first DMAs -- the
#   residual regression is the 13-filler concentration in hp=3 (DVE drain
#   backlog delays the normalization -> epilogue chain).  Every tail-
#   interleave variant measured SLOWER than the plain 30us serial tail;
#   keep False.
TAIL_INTERLEAVE = False
# v8: emit the scores matmuls one kt ahead of the pv matmuls.  The PE queue
# is strict FIFO: with program order [sc(kt), pv(kt), sc(kt+1), ...] the pv
# stalls the queue until exp(kt) completes, which delays sc(kt+1) and hence
# exp(kt+1) -- the ACT engine (the bottleneck at 1403ns per exp) sees a
# bubble every iteration.  Order [sc(kt+1), pv(kt), ...] keeps the next
# exp's input ready before the stall.  (measured neutral on hw)
SCORE_LOOKAHEAD = True
# v11: 1.5-buffered exp -- alternate a 4-bank pair tile A (fd=2048) with a
# 2-bank single tile B (fd=1024): 5 pair-exps + 6 single-exps per loop =
# 18.5us of ACT vs 16x1403=22.4us, and every refill hides under the OTHER
# buffer's exp (unlike v9's single buffer).  Bank budget A4+B2+opp1+fill1=8
# requires: PV split into sequential per-head sweeps (pt tiles buffered in
# SBUF, one [65,512] accumulator bank live at a time) and fill bufs=1.
EXP15 = True
# v9: halve the ACT instruction count by exping kt-PAIRS: one [128,2048]
# activation (785ns overhead + 0.604ns/elem ~= 2022ns) instead of two
# [128,1024] ones (2x1403ns).  The pair's scores psum tile is 4 banks --
# with opp(2) + fill(2) that is all 8, so the pair buffer is SINGLE-
# buffered: scores(pair p+1) wait for exp(p).  Predicted ~342us; MEASURED
# 497us on hw -- the exposed refill serialization costs ~1.9us/pair, far
# more than the sem+4-matmul model.  Keep OFF.
EXP_PAIRS = False


def _build(debug=False, repeats=1, loop_n=0):
    nc = bacc.Bacc("TRN2", target_bir_lowering=False, debug=False,
                   num_devices=N_CORES)
    xT = nc.dram_tensor("xT", [E, S], F32, kind="ExternalInput").ap()
    wT = nc.dram_tensor("wT", [E, 3 * FG], F32, kind="ExternalInput").ap()
    bqk = nc.dram_tensor("bqk", [2 * FG, 1], F32, kind="ExternalInput").ap()
    bv = nc.dram_tensor("bv", [1, FG], F32, kind="ExternalInput").ap()
    wpT = nc.dram_tensor("wpT", [FG, E], F32, kind="ExternalInput").ap()
    part = nc.dram_tensor("part", [S, E], F32, kind="ExternalOutput").ap()

    Exp = mybir.ActivationFunctionType.Exp

    with tile.TileContext(nc) as tc, ExitStack() as ctx:
        # ---- long-lived tiles ----
        pers = ctx.enter_context(tc.tile_pool(name="pers", bufs=1))

        vb = pers.tile([128, FG], F32, tag="vb")
        nc.sync.dma_start(out=vb, in_=bv.partition_broadcast(128))
        bqk_t = []
        for ft in range(8):
            t = pers.tile([128, 1], F32, tag=f"bqk{ft}", name=f"bqk{ft}")
            nc.sync.dma_start(out=t, in_=bqk[ft * 128:(ft + 1) * 128, :])
            bqk_t.append(t)
        # q/k feature-major tiles: ft 0..3 = q features, 4..7 = k features
        qkT = [pers.tile([128, S], BF16, tag=f"qk{ft}", name=f"qk{ft}")
               for ft in range(8)]
        # v_ext: [s-tile, (8 heads x (64 v cols + ones col))]
        ones = pers.tile([128, HG], F32, tag="ones")
        nc.vector.memset(ones, 1.0)
        vx = []
        for st in range(ST):
            t = pers.tile([128, HG * (DH + 1)], BF16, tag=f"vx{st}",
                          name=f"vx{st}")
            nc.vector.tensor_copy(
                t.rearrange("p (h c) -> p h c", c=DH + 1)[:, :, DH], ones)
            vx.append(t)
        # weights resident in bf16 (cast during DMA on the gpsimd queue);
        # q|k|v slices merged into one tile per e-tile => one DMA each
        wqkv_t = [pers.tile([128, 3 * FG], BF16, tag=f"wqkv{e}",
                            name=f"wqkv{e}") for e in range(ET)]
        wpT_t = [pers.tile([128, E], BF16, tag=f"wp{et}", name=f"wp{et}")
                 for et in range(4)]
        # x resident in bf16, reloaded once per rep
        xb = [pers.tile([128, S], BF16, tag=f"xb{e}", name=f"xb{e}")
              for e in range(ET)]


        fill = ctx.enter_context(tc.tile_pool(name="fill",
                                              bufs=1 if EXP15 else 2,
                                              space="PSUM"))
        # EXP_PAIRS: one [128,2048] pair tile = 4 banks; with opp(2)+fill(2)
        # that is all 8 PSUM banks, so the pair buffer cannot double-buffer
        if EXP15:
            scpA = ctx.enter_context(tc.tile_pool(name="scpA", bufs=1,
                                                  space="PSUM"))
            scpB = ctx.enter_context(tc.tile_pool(name="scpB", bufs=1,
                                                  space="PSUM"))
            scp = None
        else:
            scp = ctx.enter_context(tc.tile_pool(name="scp",
                                                 bufs=1 if EXP_PAIRS else 2,
                                                 space="PSUM"))
        opp = ctx.enter_context(tc.tile_pool(name="opp",
                                             bufs=1 if EXP15 else 2,
                                             space="PSUM"))
        ptp = ctx.enter_context(tc.tile_pool(name="ptp", bufs=4))
        aocp = ctx.enter_context(tc.tile_pool(name="aocp", bufs=2))
        drnp = ctx.enter_context(tc.tile_pool(name="drnp", bufs=3))
        recp = ctx.enter_context(tc.tile_pool(name="recp", bufs=2))
        outp = ctx.enter_context(tc.tile_pool(name="outp", bufs=2))
        rbp = ctx.enter_context(tc.tile_pool(name="rbp", bufs=2))

        # ---------- emission helpers ----------
        def inputs_dma_wx(x_first=False):
            order = ["x", "w"] if x_first else ["w", "x"]
            for which in order:
                for e in range(ET):
                    if which == "w":
                        nc.gpsimd.dma_start(
                            out=wqkv_t[e], in_=wT[e * 128:(e + 1) * 128, :])
                    else:
                        nc.gpsimd.dma_start(
                            out=xb[e], in_=xT[e * 128:(e + 1) * 128, :])

        def inputs_dma_wp():
            for et in range(4):
                nc.gpsimd.dma_start(
                    out=wpT_t[et], in_=wpT[et * 128:(et + 1) * 128, :])

        def inputs_dma():
            inputs_dma_wx()
            inputs_dma_wp()

        def kf_piece(sq, kf):
            s0 = sq * 512
            ft = 4 + kf
            pp = fill.tile([128, 512], F32, tag="fl",
                           name=f"kp{sq}_{kf}")
            for e in range(ET):
                nc.tensor.matmul(
                    pp, lhsT=wqkv_t[e][:, FG + kf * 128:FG + (kf + 1) * 128],
                    rhs=xb[e][:, s0:s0 + 512], start=(e == 0),
                    stop=(e == ET - 1))
            nc.vector.tensor_scalar_add(
                out=qkT[ft][:, s0:s0 + 512], in0=pp,
                scalar1=bqk_t[ft])

        def v_piece(st):
            sq, sl = st // 4, st % 4
            s0 = sq * 512
            pp = fill.tile([128, FG], F32, tag="fl",
                           name=f"vp{st}")
            for e in range(ET):
                nc.tensor.matmul(
                    pp, lhsT=xb[e][:, s0 + sl * 128:s0 + (sl + 1) * 128],
                    rhs=wqkv_t[e][:, 2 * FG:3 * FG],
                    start=(e == 0), stop=(e == ET - 1))
            nc.vector.tensor_add(
                out=vx[st].rearrange("p (h c) -> p h c",
                                     c=DH + 1)[:, :, 0:DH],
                in0=pp.rearrange("p (h c) -> p h c", c=DH),
                in1=vb.rearrange("p (h c) -> p h c", c=DH))

        def kv_quarter(sq):
            for kf in range(4):
                kf_piece(sq, kf)
            for sl in range(4):
                v_piece(sq * 4 + sl)

        def q_proj_half(qc, ft, half):
            """4 of the 8 contraction matmuls for one q feature tile."""
            s0 = qc * 512
            pp = fill.tile([128, 512], F32, tag="fl",
                           name=f"qp{qc}_{ft}_{half}")
            es = range(0, 4) if half == 0 else range(4, ET)
            for e in es:
                nc.tensor.matmul(
                    pp, lhsT=wqkv_t[e][:, ft * 128:(ft + 1) * 128],
                    rhs=xb[e][:, s0:s0 + 512],
                    start=(e == es.start), stop=False)
            if half == 0:
                return pp
            nc.vector.tensor_scalar_add(
                out=qkT[ft][:, s0:s0 + 512], in0=pp, scalar1=bqk_t[ft])
            return None

        def q_proj_full(qc, ft):
            s0 = qc * 512
            pp = fill.tile([128, 512], F32, tag="fl",
                           name=f"qpf{qc}_{ft}")
            for e in range(ET):
                nc.tensor.matmul(
                    pp, lhsT=wqkv_t[e][:, ft * 128:(ft + 1) * 128],
                    rhs=xb[e][:, s0:s0 + 512],
                    start=(e == 0), stop=(e == ET - 1))
            nc.vector.tensor_scalar_add(
                out=qkT[ft][:, s0:s0 + 512], in0=pp, scalar1=bqk_t[ft])

        def out_proj(qc_prev, sl, aoc_prev):
            st = qc_prev * 4 + sl
            c0 = st * 128
            lo = sl * 128
            ot = outp.tile([128, E], F32, tag="ot")
            for fc in range(2):
                f0 = fc * 512
                pp = fill.tile([128, 512], F32, tag="fl",
                               name=f"pj{st}_{fc}")
                for et in range(4):
                    nc.tensor.matmul(
                        pp, lhsT=aoc_prev[et][:, lo:lo + 128],
                        rhs=wpT_t[et][:, f0:f0 + 512],
                        start=(et == 0), stop=(et == 3))
                nc.vector.tensor_copy(ot[:, f0:f0 + 512], pp)
            nc.sync.dma_start(out=part[c0:c0 + 128, :], in_=ot)

        # prologue: first iteration's inputs + its k-quarter-0 and qc0
        # q-projection, emitted once ahead of the loop (each iteration's
        # tail then produces these for the next iteration)
        inputs_dma()
        kv_quarter(0)
        for ft in range(4):
            q_proj_full(0, ft)

        import contextlib
        rep_ctx = (tc.For_i(0, loop_n, 1, name="bench")
                   if loop_n else contextlib.nullcontext())
        with rep_ctx:
          for _rep in range(repeats):

            # ---------- emission ----------
            if not INTERLEAVE_KV:
                for sq in range(1, 4):
                    kv_quarter(sq)

            aoc_prev = None
            for qc in range(4):
                q0 = qc * 512
                if qc == 3 and TAIL_INTERLEAVE == 1:
                    # next rep's w_qkv/x casting DMAs: started here so the
                    # tail compute (interleaved into hp=3 below) has its
                    # inputs ~20us before it needs them.  wpT DMAs stay at
                    # the rep tail: the qc=3 out-projection still reads wpT_t.
                    inputs_dma_wx()
                aoc_cur = [aocp.tile([128, 512], BF16, tag=f"aoc{et}",
                                     name=f"aoc{qc}_{et}")
                           for et in range(4)]
                for hp in range(4):
                    # PE filler pieces to emit inside the kt loop, keyed by kt
                    filler = {}
                    if qc == 0 and hp == 0:
                        # overlap remaining k/v quarters with the first
                        # head-pair's attention sweep.  Under EXP15 the
                        # group loop emits scores one GROUP ahead, so each
                        # kv quarter must precede the scores of its first
                        # kt by two groups: kt 1/5/9 (vs 3/7/11).
                        if INTERLEAVE_KV:
                            ks = (1, 5, 9) if EXP15 else (3, 7, 11)
                            filler[ks[0]] = lambda: kv_quarter(1)
                            filler[ks[1]] = lambda: kv_quarter(2)
                            filler[ks[2]] = lambda: kv_quarter(3)
                    elif qc == 3 and hp == 3 and TAIL_INTERLEAVE:
                        # software-pipelined rep boundary: next rep's
                        # kv-quarter-0 and q-projection(qc0) interleave into
                        # the last head-pair's attention sweep (slots chosen
                        # WAR-safe: kf3 after this hp's scores pass kt=3,
                        # v(st) after PV(kt=st))
                        filler[1] = (kf_piece, (0, 0))
                        filler[2] = (out_proj, (2, 3, aoc_prev))
                        filler[3] = (kf_piece, (0, 1))
                        filler[4] = (kf_piece, (0, 2))
                        filler[5] = (kf_piece, (0, 3))
                        filler[6] = (v_piece, (0,))
                        filler[7] = (q_proj_full, (0, 0))
                        filler[8] = (v_piece, (1,))
                        filler[9] = (q_proj_full, (0, 1))
                        filler[10] = (v_piece, (2,))
                        filler[11] = (q_proj_full, (0, 2))
                        filler[12] = (v_piece, (3,))
                        filler[13] = (q_proj_full, (0, 3))
                    else:
                        pieces = []
                        if qc < 3:
                            if qc == 0:
                                # 4 q feature tiles over head-pairs 1..3
                                fts = {1: [0], 2: [1], 3: [2, 3]}[hp]
                                for ft in fts:
                                    if QPROJ_HALVES:
                                        pieces.append(
                                            (q_proj_half, (qc + 1, ft, 0)))
                                        pieces.append(
                                            (q_proj_half, (qc + 1, ft, 1)))
                                    else:
                                        pieces.append(
                                            (q_proj_full, (qc + 1, ft)))
                            elif QPROJ_HALVES:
                                pieces.append((q_proj_half, (qc + 1, hp, 0)))
                                pieces.append((q_proj_half, (qc + 1, hp, 1)))
                            else:
                                pieces.append((q_proj_full, (qc + 1, hp)))
                        if qc > 0:
                            pieces.append((out_proj, (qc - 1, hp, aoc_prev)))
                        slots = [2, 5, 8, 11][:len(pieces)]
                        for s, p in zip(slots, pieces):
                            filler[s] = (p[0], p[1])

                    qTt, kTt = qkT[hp], qkT[4 + hp]
                    if EXP15:
                        # sequential per-head accumulators (1 bank live)
                        ops = [None, None]
                        ops[0] = opp.tile([DH + 1, 512], F32, tag="op",
                                          name=f"op{hp}_{qc}_0")
                    else:
                        ops = []
                        for hh in range(2):
                            op = opp.tile([DH + 1, 512], F32, tag="op",
                                          name=f"op{hp}_{qc}_{hh}")
                            ops.append(op)
                    half_pp = None

                    def emit_filler(kt):
                        nonlocal half_pp
                        f = filler.pop(kt, None)
                        if f is None:
                            return
                        if callable(f):
                            f()
                            return
                        fn, args = f
                        if fn is q_proj_half:
                            if args[2] == 0:
                                half_pp = fn(*args)
                            else:
                                # second half continues on half_pp
                                qc_, ft_, _ = args
                                s0_ = qc_ * 512
                                for e in range(4, ET):
                                    nc.tensor.matmul(
                                        half_pp,
                                        lhsT=wqkv_t[e][:, ft_ * 128:
                                                       (ft_ + 1) * 128],
                                        rhs=xb[e][:, s0_:s0_ + 512],
                                        start=False,
                                        stop=(e == ET - 1))
                                nc.vector.tensor_scalar_add(
                                    out=qkT[ft_][:, s0_:s0_ + 512],
                                    in0=half_pp,
                                    scalar1=bqk_t[ft_])
                                half_pp = None
                        else:
                            fn(*args)

                    def score_mms(sc, col0, kt):
                        k0 = kt * 128
                        for hh in range(2):
                            r = slice(hh * DH, (hh + 1) * DH)
                            nc.tensor.matmul(
                                sc[:, col0 + hh * 512:col0 + (hh + 1) * 512],
                                lhsT=kTt[r, k0:k0 + 128],
                                rhs=qTt[r, q0:q0 + 512],
                                start=True, stop=True)

                    def pv_mms(pt, col0, kt):
                        for hh in range(2):
                            h = hp * 2 + hh
                            nc.tensor.matmul(
                                ops[hh],
                                lhsT=vx[kt][:, h * (DH + 1):
                                            (h + 1) * (DH + 1)],
                                rhs=pt[:, col0 + hh * 512:
                                        col0 + (hh + 1) * 512],
                                start=(kt == 0), stop=(kt == ST - 1))

                    if EXP15:
                        # 1.5-buffered exp: groups alternate the 4-bank A
                        # pair tile (fd=2048) and the 2-bank B tile
                        # (fd=1024); the next group's scores are emitted
                        # right after this group's exp, so every refill
                        # hides under the other buffer's exp.  PV hh=0 runs
                        # in-loop; hh=1 sweeps afterwards from the buffered
                        # pt tiles (ops bank freed by then).
                        GROUPS = [("A", (0, 1)), ("B", (2,)),
                                  ("A", (3, 4)), ("B", (5,)),
                                  ("A", (6, 7)), ("B", (8,)),
                                  ("A", (9, 10)), ("B", (11,)),
                                  ("A", (12, 13)), ("B", (14,)),
                                  ("B", (15,))]

                        def pv_one(op, hh, pt, col0, kt):
                            h = hp * 2 + hh
                            nc.tensor.matmul(
                                op,
                                lhsT=vx[kt][:, h * (DH + 1):
                                            (h + 1) * (DH + 1)],
                                rhs=pt[:, col0 + hh * 512:
                                        col0 + (hh + 1) * 512],
                                start=(kt == 0), stop=(kt == ST - 1))

                        def group_scores(g):
                            pool_, kts_ = GROUPS[g]
                            p = scpA if pool_ == "A" else scpB
                            sc = p.tile([128, 1024 * len(kts_)], F32,
                                        tag="sc", name=f"sc{hp}_{qc}_g{g}")
                            for j, kt in enumerate(kts_):
                                score_mms(sc, j * 1024, kt)
                            return sc

                        pts = []
                        sc_cur = group_scores(0)
                        for g, (pool_, kts_) in enumerate(GROUPS):
                            pt = ptp.tile([128, 1024 * len(kts_)], BF16,
                                          tag=f"pt{pool_}",
                                          bufs=6 if pool_ == "A" else 8,
                                          name=f"pt{hp}_{qc}_g{g}")
                            nc.scalar.activation(pt, sc_cur, Exp,
                                                 scale=0.125)
                            if g + 1 < len(GROUPS):
                                sc_cur = group_scores(g + 1)
                            for j, kt in enumerate(kts_):
                                pts.append((pt, j * 1024, kt))
                                pv_one(ops[0], 0, pt, j * 1024, kt)
                                emit_filler(kt)
                        # hh=1 sweep from the buffered pt tiles; its PE work
                        # hides under the next loop's exps
                        ops[1] = opp.tile([DH + 1, 512], F32, tag="op",
                                          name=f"op{hp}_{qc}_1")
                        for (pt, c0, kt) in pts:
                            pv_one(ops[1], 1, pt, c0, kt)
                    elif EXP_PAIRS:
                        # one [128,2048] exp per kt-PAIR; the 4-bank pair
                        # psum is single-buffered (all 8 banks committed).
                        # PE-queue order per pair (FIFO!): fillers run while
                        # the ACT exps, the NEXT pair's scores go before the
                        # pv so the ACT restarts after only ~4 matmuls, and
                        # the pv (which only needs pt in SBUF) trails.
                        def pair_scores(kp):
                            sc = scp.tile([128, 2048], F32, tag="sc",
                                          name=f"sc{hp}_{qc}_{kp}")
                            for j in range(2):
                                score_mms(sc, j * 1024, 2 * kp + j)
                            return sc

                        sc_cur = pair_scores(0)
                        for kp in range(ST // 2):
                            pt = ptp.tile([128, 2048], BF16, tag="pt",
                                          name=f"pt{hp}_{qc}_{kp}")
                            nc.scalar.activation(pt, sc_cur, Exp, scale=0.125)
                            emit_filler(2 * kp)
                            emit_filler(2 * kp + 1)
                            if kp + 1 < ST // 2:
                                sc_cur = pair_scores(kp + 1)
                            for j in range(2):
                                pv_mms(pt, j * 1024, 2 * kp + j)
                    else:
                        # lookahead is ILLEGAL in the (qc0, hp0) loop: its
                        # fillers (kv quarters 1-3) produce the k-features
                        # that scores(kt+1) consumes -- hoisting the scores
                        # above the filler would read stale qkT
                        la = SCORE_LOOKAHEAD and not (
                            qc == 0 and hp == 0 and INTERLEAVE_KV)
                        sc_next = None
                        for kt in range(ST):
                            if sc_next is None:
                                sc_next = scp.tile([128, 1024], F32,
                                                   tag="sc",
                                                   name=f"sc{hp}_{qc}_{kt}")
                                score_mms(sc_next, 0, kt)
                            sc = sc_next
                            sc_next = None
                            if la and kt + 1 < ST:
                                sc_next = scp.tile([128, 1024], F32,
                                                   tag="sc",
                                                   name=f"sc{hp}_{qc}_{kt+1}")
                                score_mms(sc_next, 0, kt + 1)
                            pt = ptp.tile([128, 1024], BF16, tag="pt",
                                          name=f"pt{hp}_{qc}_{kt}")
                            nc.scalar.activation(pt, sc, Exp, scale=0.125)
                            pv_mms(pt, 0, kt)
                            emit_filler(kt)
                    # any unemitted filler (shouldn't happen, but be safe)
                    for kt in sorted(filler):
                        f = filler[kt]
                        if callable(f):
                            f()
                        else:
                            fn, args = f
                            fn(*args)
                    for hh in range(2):
                        # one DVE copy drains the psum accumulator (freeing
                        # the bank); normalization runs from SBUF
                        drn = drnp.tile([DH + 1, 512], F32, tag="drn",
                                        name=f"drn{hp}_{qc}_{hh}")
                        nc.vector.tensor_copy(drn, ops[hh])
                        srow = recp.tile([1, 512], F32, tag="srow")
                        nc.vector.tensor_copy(srow, drn[DH:DH + 1, :])
                        rec = recp.tile([1, 512], F32, tag="rec")
                        nc.vector.reciprocal_approx_fast(rec, srow)
                        rb = rbp.tile([DH, 512], F32, tag="rb")
                        nc.gpsimd.partition_broadcast(rb, rec)
                        nc.vector.tensor_mul(
                            out=aoc_cur[hp][hh * DH:(hh + 1) * DH, :],
                            in0=drn[0:DH, :], in1=rb)
                aoc_prev = aoc_cur
            if TAIL_INTERLEAVE == 2:
                # x/w DMAs AFTER the qc=3 loops (so the stale-reading hp=3
                # fillers never wait on them) but BEFORE the epilogue, x
                # first: the next rep's kv-quarter-1 filler (its earliest
                # DMA consumer, ~10us away) waits only on the 8 x tiles.
                inputs_dma_wx(x_first=True)
            # epilogue: out projection for the last chunk.  With EXP_PAIRS the
            # scp pair tile is single-buffered, so run the epilogue through
            # the fill pool (2 bufs) to keep two tiles in flight; otherwise
            # use the (now idle) scp banks as before.
            if EXP_PAIRS or EXP15:
                for sl in range(4):
                    st = 3 * 4 + sl
                    c0 = st * 128
                    lo = sl * 128
                    ot = outp.tile([128, E], F32, tag="ot")
                    for fc in range(2):
                        f0 = fc * 512
                        pp = fill.tile([128, 512], F32, tag="fl",
                                       name=f"ep{sl}_{fc}")
                        for et in range(4):
                            nc.tensor.matmul(
                                pp,
                                lhsT=aoc_prev[et][:, lo:lo + 128],
                                rhs=wpT_t[et][:, f0:f0 + 512],
                                start=(et == 0), stop=(et == 3))
                        nc.vector.tensor_copy(ot[:, f0:f0 + 512], pp)
                    nc.sync.dma_start(out=part[c0:c0 + 128, :], in_=ot)
            else:
                for sl in range(4):
                    st = 3 * 4 + sl
                    c0 = st * 128
                    lo = sl * 128
                    pp = scp.tile([128, 1024], F32, tag="sc", name=f"ep{sl}")
                    for fc in range(2):
                        f0 = fc * 512
                        for et in range(4):
                            nc.tensor.matmul(
                                pp[:, f0:f0 + 512],
                                lhsT=aoc_prev[et][:, lo:lo + 128],
                                rhs=wpT_t[et][:, f0:f0 + 512],
                                start=(et == 0), stop=(et == 3))
                    ot = outp.tile([128, E], F32, tag="ot")
                    nc.vector.tensor_copy(ot, pp)
                    nc.sync.dma_start(out=part[c0:c0 + 128, :], in_=ot)

            # prefetch the next iteration's inputs and precompute its
            # k-quarter-0 / q-projection(qc0) so the next iteration opens
            # directly with score matmuls (software-pipelined loop boundary)
            if TAIL_INTERLEAVE == 1:
                # kv-quarter-0 / q-proj(0) already ran as hp=3 fillers;
                # only the out-projection weights remain (WAR vs epilogue)
                inputs_dma_wp()
            elif TAIL_INTERLEAVE == 2:
                # x/w DMAs already issued before the epilogue; only the
                # out-projection weights remain (WAR vs the epilogue reads)
                inputs_dma_wp()
            else:
                inputs_dma()
                kv_quarter(0)
                for ft in range(4):
                    q_proj_full(0, ft)

    nc.compile()
    return nc


def _get_runner(debug=False, repeats=1, loop_n=0):
    """Build (once) a cached jit'd SPMD runner over the 8 axon cores."""
    key = ("run", debug, repeats, loop_n)
    if key in _CACHE:
        return _CACHE[key]

    import jax
    from jax.experimental.shard_map import shard_map
    from jax.sharding import Mesh, PartitionSpec, NamedSharding
    from concourse.bass2jax import (install_neuronx_cc_hook, _bass_exec_p,
                                    partition_id_tensor)

    nc = _build(debug, repeats, loop_n)
    install_neuronx_cc_hook()

    in_names, out_names, out_avals, zero_outs = [], [], [], []
    partition_name = nc.partition_id_tensor.name if nc.partition_id_tensor else None
    for alloc in nc.m.functions[0].allocations:
        if not isinstance(alloc, mybir.MemoryLocationSet):
            continue
        name = alloc.memorylocations[0].name
        if alloc.kind == "ExternalInput":
            if name != partition_name:
                in_names.append(name)
        elif alloc.kind == "ExternalOutput":
            shape = tuple(alloc.tensor_shape)
            dtype = mybir.dt.np(alloc.dtype)
            out_names.append(name)
            out_avals.append(jax.core.ShapedArray(shape, dtype))
            zero_outs.append(np.zeros(shape, dtype))
    n_params = len(in_names)
    n_outs = len(out_names)
    all_in_names = in_names + out_names
    if partition_name is not None:
        all_in_names.append(partition_name)

    def _body(*args):
        operands = list(args)
        if partition_name is not None:
            operands.append(partition_id_tensor())
        outs = _bass_exec_p.bind(
            *operands,
            out_avals=tuple(out_avals),
            in_names=tuple(all_in_names),
            out_names=tuple(out_names),
            lowering_input_output_aliases=(),
            sim_require_finite=True,
            sim_require_nnan=True,
            nc=nc,
        )
        return tuple(outs)

    devices = jax.devices()[:N_CORES]
    mesh = Mesh(np.asarray(devices), ("core",))
    in_specs = (PartitionSpec("core"),) * (n_params + n_outs)
    out_specs = (PartitionSpec("core"),) * n_outs
    sharded = jax.jit(
        shard_map(_body, mesh=mesh, in_specs=in_specs, out_specs=out_specs,
                  check_rep=False),
        donate_argnums=tuple(range(n_params, n_params + n_outs)),
        keep_unused=True,
    )
    sharded_nodonate = jax.jit(
        shard_map(_body, mesh=mesh, in_specs=in_specs, out_specs=out_specs,
                  check_rep=False),
        keep_unused=True,
    )
    core_sharding = NamedSharding(mesh, PartitionSpec("core"))

    def run(in_maps, timing_iters=0):
        concat_in = [
            np.concatenate([np.asarray(m[name]) for m in in_maps], axis=0)
            for name in in_names
        ]
        concat_zeros = [
            np.zeros((N_CORES * z.shape[0], *z.shape[1:]), z.dtype)
            for z in zero_outs
        ]
        out_arrs = sharded(*concat_in, *concat_zeros)
        results = [
            {name: np.asarray(out_arrs[i]).reshape(N_CORES, *out_avals[i].shape)[c]
             for i, name in enumerate(out_names)}
            for c in range(N_CORES)
        ]
        times = []
        if timing_iters:
            import time
            dev = [jax.device_put(a, core_sharding)
                   for a in concat_in + concat_zeros]
            jax.block_until_ready(dev)
            for _ in range(2):
                jax.block_until_ready(sharded_nodonate(*dev))
            for _ in range(timing_iters):
                t0 = time.perf_counter()
                jax.block_until_ready(sharded_nodonate(*dev))
                times.append(time.perf_counter() - t0)
        return results, times

    _CACHE[key] = run
    return run


def _shard_inputs(x, w_qkv, b_qkv, w_proj):
    x = np.asarray(x, np.float32)
    w = np.asarray(w_qkv, np.float32)
    bq = np.asarray(b_qkv, np.float32)
    wp = np.asarray(w_proj, np.float32)
    in_maps = []
    for b in range(B):
        xTb = np.ascontiguousarray(x[b].T)                      # [E, S]
        for g in range(2):
            r = slice(g * FG, (g + 1) * FG)
            w_slice = np.concatenate([w[0:E][r], w[E:2 * E][r],
                                      w[2 * E:3 * E][r]], axis=0)  # [1536, E]
            in_maps.append({
                "xT": xTb,
                "wT": np.ascontiguousarray(w_slice.T),          # [E, 1536]
                "bqk": np.concatenate([bq[0:E][r], bq[E:2 * E][r]]
                                      ).reshape(2 * FG, 1).astype(np.float32),
                "bv": bq[2 * E:3 * E][r].reshape(1, FG).astype(np.float32),
                "wpT": np.ascontiguousarray(wp[:, r].T),        # [FG, E]
            })
    return in_maps


def _gather(results, b_proj):
    bp = np.asarray(b_proj, np.float32)
    out = np.empty((B, S, E), np.float32)
    for b in range(B):
        out[b] = results[2 * b]["part"] + results[2 * b + 1]["part"] + bp
    return out


def kernel(x, w_qkv, b_qkv, w_proj, b_proj):
    run = _get_runner()
    in_maps = _shard_inputs(x, w_qkv, b_qkv, w_proj)
    results, _ = run(in_maps)
    return _gather(results, b_proj)


def kernel_timed(x, w_qkv, b_qkv, w_proj, b_proj, iters=5):
    """Like kernel() but also returns per-call device wall times (seconds)."""
    run = _get_runner()
    in_maps = _shard_inputs(x, w_qkv, b_qkv, w_proj)
    results, times = run(in_maps, timing_iters=iters)
    return _gather(results, b_proj), times


def device_time_ns(inputs, loop_n=129, iters=20, rounds=5):
    """Device execution time per kernel invocation (ns), via hardware-loop
    delta: wall(loop_n=N) - wall(loop_n=1) = (N-1) * T_device.  Cancels the
    host/RPC dispatch overhead (~70-140 ms through the axon tunnel), which
    dominates single-call wall time.  Each round pairs a loop_n=1 and a
    loop_n=N measurement under the same network conditions; the median of
    per-round deltas rejects outlier rounds."""
    in_maps = _shard_inputs(inputs["x"], inputs["w_qkv"], inputs["b_qkv"],
                            inputs["w_proj"])
    r1 = _get_runner(loop_n=0)
    rN = _get_runner(loop_n=loop_n)
    deltas = []
    for _ in range(rounds):
        _, t1 = r1(in_maps, timing_iters=iters)
        _, tN = rN(in_maps, timing_iters=iters)
        deltas.append((min(tN) - min(t1)) / (loop_n - 1) * 1e9)
    deltas.sort()
    # lower-median: drift only ever inflates a round, never deflates it
    return deltas[(len(deltas) - 1) // 2]



# revision 41
# speedup vs baseline: 1.1787x; 1.1787x over previous
"""Multi-head attention (B=4, S=2048, E=1024, H=16, Dh=64) on 8 TRN2 NeuronCores.

Sharding: data-parallel over batch (4) x tensor-parallel over head-groups (2).
Core (b, g) computes heads g*8 .. g*8+7 of batch b end-to-end: qkv projection,
attention, and the output-projection partial sum over its 512 attention-output
features.  The host sums the two per-batch partials and adds b_proj.

v4 (vs the fp32r baseline):
  - all matmul operands bf16 (fp32 PSUM accumulate); inputs cast on device
    via gpsimd casting DMAs.  exp writes bf16.  ACT runs exp only.
  - x loaded once (resident bf16 [8][128, 2048]); all inputs arrive as 20
    full-tile casting DMAs per iteration (q|k|v weight slices merged into one
    [128,1536] tile per e-tile), prefetched at the previous iteration's tail.
  - phase A (k/v projection) quarter-interleaved into the first head-pair's
    attention k-loop, so the scalar engine starts exp ~25us into the kernel
    instead of ~90us.
  - q-proj / out-proj PE work emitted as small pieces inside the attention
    k-loop (instead of ahead of each head-pair) to keep ACT fed.
  - attention-output psum is drained to SBUF with one DVE copy per head so
    the psum bank frees ~2us earlier; normalization runs from SBUF.

v8 session notes (all numbers hw-measured via loop-delta microbenches):
  - the kernel is ACT-bound: exp [128,1024] psum->sbuf costs 1403ns back to
    back (785ns fixed overhead + 0.604ns/elem), so 256 exps = 359us of the
    ~405us kernel.  bf16 matmuls N=512 stream at 110ns (2 cols/cycle), so
    the PE (1632 MMs) is only ~180us busy -- NOT the bottleneck.
  - exp free-dim is capped at 1024 by PSUM: scp 2x[128,1024] (4 banks) +
    ops 2x[65,512] (2) + fill 2x[128,512] (2) = all 8 banks.
  - things measured SLOWER and reverted: Schraudolph-on-DVE exp offload of
    4/16 kt tiles (444us plain / 420us with high_priority+retuned kts; the
    cross-engine coupling on the sc double-buffer eats the ACT savings);
    interleaving the rep tail into qc=3 (422us); fp8 PV fails precision
    (3.7e-2 > 2e-2: softmax concentrates on large p, errors don't average).
  - SCORE_LOOKAHEAD (scores one kt ahead of pv) is neutral (405us) -- the
    Tile scheduler already reorders around the exp->pv stall.
"""

import numpy as np
from contextlib import ExitStack

import concourse.bacc as bacc
import concourse.bass as bass
import concourse.tile as tile
import concourse.mybir as mybir

B, S, E, H, DH = 4, 2048, 1024, 16, 64
N_CORES = 8
FG = 512          # features per head-group (8 heads x 64)
HG = 8            # heads per core
ET = E // 128     # 8 e-tiles (qkv contraction)
ST = S // 128     # 16 s-tiles
F32 = mybir.dt.float32
F32R = mybir.dt.float32r
BF16 = mybir.dt.bfloat16

_CACHE: dict = {}
INTERLEAVE_KV = True
QPROJ_HALVES = True
ROTATE_DMA = True
# v7: software-pipeline the rep boundary INTO the last head-pair's attention
# sweep: next rep's kv-quarter-0 + q-proj(qc0) run as qc=3/hp=3 fillers.
# Mode 1: the w/x casting DMAs also move to qc=3's head so the fillers read
#   fresh data.  (measured: -17us SLOWER on hw -- the 20 casting DMAs
#   contend on the Pool queue during qc=3 and the wpT WAR serializes)
# Mode 2: fillers read the CURRENT resident w/x tiles (a WAR against the
#   tail DMAs, which never stalls the readers).  The weights are constant
#   and x is rep-invariant in the timing loop, so the values are identical.
#   Measured 443us with tail DMAs (next rep's kv-quarter-1 stalls on the
#   20-DMA Pool chain) and 444us with pre-epilogue x-first DMAs -- the
#   residual regression is the 13-filler concentration in hp=3 (DVE drain
#   backlog delays the normalization -> epilogue chain).  Every tail-
#   interleave variant measured SLOWER than the plain 30us serial tail;
#   keep False.
TAIL_INTERLEAVE = False
# v8: emit the scores matmuls one kt ahead of the pv matmuls.  The PE queue
# is strict FIFO: with program order [sc(kt), pv(kt), sc(kt+1), ...] the pv
# stalls the queue until exp(kt) completes, which delays sc(kt+1) and hence
# exp(kt+1) -- the ACT engine (the bottleneck at 1403ns per exp) sees a
# bubble every iteration.  Order [sc(kt+1), pv(kt), ...] keeps the next
# exp's input ready before the stall.  (measured neutral on hw)
SCORE_LOOKAHEAD = True
# v11: 1.5-buffered exp -- alternate a 4-bank pair tile A (fd=2048) with a
# 2-bank single tile B (fd=1024): 5 pair-exps + 6 single-exps per loop =
# 18.5us of ACT vs 16x1403=22.4us, and every refill hides under the OTHER
# buffer's exp (unlike v9's single buffer).  Bank budget A4+B2+opp1+fill1=8
# requires: PV split into sequential per-head sweeps (pt tiles buffered in
# SBUF, one [65,512] accumulator bank live at a time) and fill bufs=1.
# MEASURED 476811ns vs the 409585 checkpoint (numerics exact) -- like v9,
# the hidden exp->refill->exp turnaround cost (~1-2us per buffer handoff)
# swamps the 63us ACT-overhead saving.  Keep OFF.
EXP15 = False
# v12: pre-cast x / w_qkv / w_proj to bf16 on the HOST (the kernel casts
# them to bf16 anyway; _shard_inputs already does host-side transposes).
# Halves the per-rep DMA read traffic (16.6MB fp32 -> 8.3MB bf16), which
# shortens the serial DMA chain exposed at the rep boundary.
HOST_BF16 = True
# v9: halve the ACT instruction count by exping kt-PAIRS: one [128,2048]
# activation (785ns overhead + 0.604ns/elem ~= 2022ns) instead of two
# [128,1024] ones (2x1403ns).  The pair's scores psum tile is 4 banks --
# with opp(2) + fill(2) that is all 8, so the pair buffer is SINGLE-
# buffered: scores(pair p+1) wait for exp(p).  Predicted ~342us; MEASURED
# 497us on hw -- the exposed refill serialization costs ~1.9us/pair, far
# more than the sem+4-matmul model.  Keep OFF.
EXP_PAIRS = False


def _build(debug=False, repeats=1, loop_n=0):
    nc = bacc.Bacc("TRN2", target_bir_lowering=False, debug=False,
                   num_devices=N_CORES)
    IN_DT = BF16 if HOST_BF16 else F32
    xT = nc.dram_tensor("xT", [E, S], IN_DT, kind="ExternalInput").ap()
    wT = nc.dram_tensor("wT", [E, 3 * FG], IN_DT, kind="ExternalInput").ap()
    bqk = nc.dram_tensor("bqk", [2 * FG, 1], F32, kind="ExternalInput").ap()
    bv = nc.dram_tensor("bv", [1, FG], F32, kind="ExternalInput").ap()
    wpT = nc.dram_tensor("wpT", [FG, E], IN_DT, kind="ExternalInput").ap()
    part = nc.dram_tensor("part", [S, E], F32, kind="ExternalOutput").ap()

    Exp = mybir.ActivationFunctionType.Exp

    with tile.TileContext(nc) as tc, ExitStack() as ctx:
        # ---- long-lived tiles ----
        pers = ctx.enter_context(tc.tile_pool(name="pers", bufs=1))

        vb = pers.tile([128, FG], F32, tag="vb")
        nc.sync.dma_start(out=vb, in_=bv.partition_broadcast(128))
        bqk_t = []
        for ft in range(8):
            t = pers.tile([128, 1], F32, tag=f"bqk{ft}", name=f"bqk{ft}")
            nc.sync.dma_start(out=t, in_=bqk[ft * 128:(ft + 1) * 128, :])
            bqk_t.append(t)
        # q/k feature-major tiles: ft 0..3 = q features, 4..7 = k features
        qkT = [pers.tile([128, S], BF16, tag=f"qk{ft}", name=f"qk{ft}")
               for ft in range(8)]
        # v_ext: [s-tile, (8 heads x (64 v cols + ones col))]
        ones = pers.tile([128, HG], F32, tag="ones")
        nc.vector.memset(ones, 1.0)
        vx = []
        for st in range(ST):
            t = pers.tile([128, HG * (DH + 1)], BF16, tag=f"vx{st}",
                          name=f"vx{st}")
            nc.vector.tensor_copy(
                t.rearrange("p (h c) -> p h c", c=DH + 1)[:, :, DH], ones)
            vx.append(t)
        # weights resident in bf16 (cast during DMA on the gpsimd queue);
        # q|k|v slices merged into one tile per e-tile => one DMA each
        wqkv_t = [pers.tile([128, 3 * FG], BF16, tag=f"wqkv{e}",
                            name=f"wqkv{e}") for e in range(ET)]
        wpT_t = [pers.tile([128, E], BF16, tag=f"wp{et}", name=f"wp{et}")
                 for et in range(4)]
        # x resident in bf16, reloaded once per rep
        xb = [pers.tile([128, S], BF16, tag=f"xb{e}", name=f"xb{e}")
              for e in range(ET)]


        fill = ctx.enter_context(tc.tile_pool(name="fill",
                                              bufs=1 if EXP15 else 2,
                                              space="PSUM"))
        # EXP_PAIRS: one [128,2048] pair tile = 4 banks; with opp(2)+fill(2)
        # that is all 8 PSUM banks, so the pair buffer cannot double-buffer
        if EXP15:
            scpA = ctx.enter_context(tc.tile_pool(name="scpA", bufs=1,
                                                  space="PSUM"))
            scpB = ctx.enter_context(tc.tile_pool(name="scpB", bufs=1,
                                                  space="PSUM"))
            scp = None
        else:
            scp = ctx.enter_context(tc.tile_pool(name="scp",
                                                 bufs=1 if EXP_PAIRS else 2,
                                                 space="PSUM"))
        opp = ctx.enter_context(tc.tile_pool(name="opp",
                                             bufs=1 if EXP15 else 2,
                                             space="PSUM"))
        ptp = ctx.enter_context(tc.tile_pool(name="ptp", bufs=4))
        aocp = ctx.enter_context(tc.tile_pool(name="aocp", bufs=2))
        drnp = ctx.enter_context(tc.tile_pool(name="drnp", bufs=3))
        recp = ctx.enter_context(tc.tile_pool(name="recp", bufs=2))
        outp = ctx.enter_context(tc.tile_pool(name="outp", bufs=2))
        rbp = ctx.enter_context(tc.tile_pool(name="rbp", bufs=2))

        # ---------- emission helpers ----------
        def inputs_dma_wx(x_first=False):
            order = ["x", "w"] if x_first else ["w", "x"]
            for which in order:
                for e in range(ET):
                    if which == "w":
                        nc.gpsimd.dma_start(
                            out=wqkv_t[e], in_=wT[e * 128:(e + 1) * 128, :])
                    else:
                        nc.gpsimd.dma_start(
                            out=xb[e], in_=xT[e * 128:(e + 1) * 128, :])

        def inputs_dma_wp():
            for et in range(4):
                nc.gpsimd.dma_start(
                    out=wpT_t[et], in_=wpT[et * 128:(et + 1) * 128, :])

        def inputs_dma():
            inputs_dma_wx()
            inputs_dma_wp()

        def kf_piece(sq, kf):
            s0 = sq * 512
            ft = 4 + kf
            pp = fill.tile([128, 512], F32, tag="fl",
                           name=f"kp{sq}_{kf}")
            for e in range(ET):
                nc.tensor.matmul(
                    pp, lhsT=wqkv_t[e][:, FG + kf * 128:FG + (kf + 1) * 128],
                    rhs=xb[e][:, s0:s0 + 512], start=(e == 0),
                    stop=(e == ET - 1))
            nc.vector.tensor_scalar_add(
                out=qkT[ft][:, s0:s0 + 512], in0=pp,
                scalar1=bqk_t[ft])

        def v_piece(st):
            sq, sl = st // 4, st % 4
            s0 = sq * 512
            pp = fill.tile([128, FG], F32, tag="fl",
                           name=f"vp{st}")
            for e in range(ET):
                nc.tensor.matmul(
                    pp, lhsT=xb[e][:, s0 + sl * 128:s0 + (sl + 1) * 128],
                    rhs=wqkv_t[e][:, 2 * FG:3 * FG],
                    start=(e == 0), stop=(e == ET - 1))
            nc.vector.tensor_add(
                out=vx[st].rearrange("p (h c) -> p h c",
                                     c=DH + 1)[:, :, 0:DH],
                in0=pp.rearrange("p (h c) -> p h c", c=DH),
                in1=vb.rearrange("p (h c) -> p h c", c=DH))

        def kv_quarter(sq):
            for kf in range(4):
                kf_piece(sq, kf)
            for sl in range(4):
                v_piece(sq * 4 + sl)

        def q_proj_half(qc, ft, half):
            """4 of the 8 contraction matmuls for one q feature tile."""
            s0 = qc * 512
            pp = fill.tile([128, 512], F32, tag="fl",
                           name=f"qp{qc}_{ft}_{half}")
            es = range(0, 4) if half == 0 else range(4, ET)
            for e in es:
                nc.tensor.matmul(
                    pp, lhsT=wqkv_t[e][:, ft * 128:(ft + 1) * 128],
                    rhs=xb[e][:, s0:s0 + 512],
                    start=(e == es.start), stop=False)
            if half == 0:
                return pp
            nc.vector.tensor_scalar_add(
                out=qkT[ft][:, s0:s0 + 512], in0=pp, scalar1=bqk_t[ft])
            return None

        def q_proj_full(qc, ft):
            s0 = qc * 512
            pp = fill.tile([128, 512], F32, tag="fl",
                           name=f"qpf{qc}_{ft}")
            for e in range(ET):
                nc.tensor.matmul(
                    pp, lhsT=wqkv_t[e][:, ft * 128:(ft + 1) * 128],
                    rhs=xb[e][:, s0:s0 + 512],
                    start=(e == 0), stop=(e == ET - 1))
            nc.vector.tensor_scalar_add(
                out=qkT[ft][:, s0:s0 + 512], in0=pp, scalar1=bqk_t[ft])

        def out_proj(qc_prev, sl, aoc_prev):
            st = qc_prev * 4 + sl
            c0 = st * 128
            lo = sl * 128
            ot = outp.tile([128, E], F32, tag="ot")
            for fc in range(2):
                f0 = fc * 512
                pp = fill.tile([128, 512], F32, tag="fl",
                               name=f"pj{st}_{fc}")
                for et in range(4):
                    nc.tensor.matmul(
                        pp, lhsT=aoc_prev[et][:, lo:lo + 128],
                        rhs=wpT_t[et][:, f0:f0 + 512],
                        start=(et == 0), stop=(et == 3))
                nc.vector.tensor_copy(ot[:, f0:f0 + 512], pp)
            nc.sync.dma_start(out=part[c0:c0 + 128, :], in_=ot)

        # prologue: first iteration's inputs + its k-quarter-0 and qc0
        # q-projection, emitted once ahead of the loop (each iteration's
        # tail then produces these for the next iteration)
        inputs_dma()
        kv_quarter(0)
        for ft in range(4):
            q_proj_full(0, ft)

        import contextlib
        rep_ctx = (tc.For_i(0, loop_n, 1, name="bench")
                   if loop_n else contextlib.nullcontext())
        with rep_ctx:
          for _rep in range(repeats):

            # ---------- emission ----------
            if not INTERLEAVE_KV:
                for sq in range(1, 4):
                    kv_quarter(sq)

            aoc_prev = None
            for qc in range(4):
                q0 = qc * 512
                if qc == 3 and TAIL_INTERLEAVE == 1:
                    # next rep's w_qkv/x casting DMAs: started here so the
                    # tail compute (interleaved into hp=3 below) has its
                    # inputs ~20us before it needs them.  wpT DMAs stay at
                    # the rep tail: the qc=3 out-projection still reads wpT_t.
                    inputs_dma_wx()
                aoc_cur = [aocp.tile([128, 512], BF16, tag=f"aoc{et}",
                                     name=f"aoc{qc}_{et}")
                           for et in range(4)]
                for hp in range(4):
                    # PE filler pieces to emit inside the kt loop, keyed by kt
                    filler = {}
                    if qc == 0 and hp == 0:
                        # overlap remaining k/v quarters with the first
                        # head-pair's attention sweep.  Under EXP15 the
                        # group loop emits scores one GROUP ahead, so each
                        # kv quarter must precede the scores of its first
                        # kt by two groups: kt 1/5/9 (vs 3/7/11).
                        if INTERLEAVE_KV:
                            ks = (1, 5, 9) if EXP15 else (3, 7, 11)
                            filler[ks[0]] = lambda: kv_quarter(1)
                            filler[ks[1]] = lambda: kv_quarter(2)
                            filler[ks[2]] = lambda: kv_quarter(3)
                    elif qc == 3 and hp == 3 and TAIL_INTERLEAVE:
                        # software-pipelined rep boundary: next rep's
                        # kv-quarter-0 and q-projection(qc0) interleave into
                        # the last head-pair's attention sweep (slots chosen
                        # WAR-safe: kf3 after this hp's scores pass kt=3,
                        # v(st) after PV(kt=st))
                        filler[1] = (kf_piece, (0, 0))
                        filler[2] = (out_proj, (2, 3, aoc_prev))
                        filler[3] = (kf_piece, (0, 1))
                        filler[4] = (kf_piece, (0, 2))
                        filler[5] = (kf_piece, (0, 3))
                        filler[6] = (v_piece, (0,))
                        filler[7] = (q_proj_full, (0, 0))
                        filler[8] = (v_piece, (1,))
                        filler[9] = (q_proj_full, (0, 1))
                        filler[10] = (v_piece, (2,))
                        filler[11] = (q_proj_full, (0, 2))
                        filler[12] = (v_piece, (3,))
                        filler[13] = (q_proj_full, (0, 3))
                    else:
                        pieces = []
                        if qc < 3:
                            if qc == 0:
                                # 4 q feature tiles over head-pairs 1..3
                                fts = {1: [0], 2: [1], 3: [2, 3]}[hp]
                                for ft in fts:
                                    if QPROJ_HALVES:
                                        pieces.append(
                                            (q_proj_half, (qc + 1, ft, 0)))
                                        pieces.append(
                                            (q_proj_half, (qc + 1, ft, 1)))
                                    else:
                                        pieces.append(
                                            (q_proj_full, (qc + 1, ft)))
                            elif QPROJ_HALVES:
                                pieces.append((q_proj_half, (qc + 1, hp, 0)))
                                pieces.append((q_proj_half, (qc + 1, hp, 1)))
                            else:
                                pieces.append((q_proj_full, (qc + 1, hp)))
                        if qc > 0:
                            pieces.append((out_proj, (qc - 1, hp, aoc_prev)))
                        slots = [2, 5, 8, 11][:len(pieces)]
                        for s, p in zip(slots, pieces):
                            filler[s] = (p[0], p[1])

                    qTt, kTt = qkT[hp], qkT[4 + hp]
                    if EXP15:
                        # sequential per-head accumulators (1 bank live)
                        ops = [None, None]
                        ops[0] = opp.tile([DH + 1, 512], F32, tag="op",
                                          name=f"op{hp}_{qc}_0")
                    else:
                        ops = []
                        for hh in range(2):
                            op = opp.tile([DH + 1, 512], F32, tag="op",
                                          name=f"op{hp}_{qc}_{hh}")
                            ops.append(op)
                    half_pp = None

                    def emit_filler(kt):
                        nonlocal half_pp
                        f = filler.pop(kt, None)
                        if f is None:
                            return
                        if callable(f):
                            f()
                            return
                        fn, args = f
                        if fn is q_proj_half:
                            if args[2] == 0:
                                half_pp = fn(*args)
                            else:
                                # second half continues on half_pp
                                qc_, ft_, _ = args
                                s0_ = qc_ * 512
                                for e in range(4, ET):
                                    nc.tensor.matmul(
                                        half_pp,
                                        lhsT=wqkv_t[e][:, ft_ * 128:
                                                       (ft_ + 1) * 128],
                                        rhs=xb[e][:, s0_:s0_ + 512],
                                        start=False,
                                        stop=(e == ET - 1))
                                nc.vector.tensor_scalar_add(
                                    out=qkT[ft_][:, s0_:s0_ + 512],
                                    in0=half_pp,
                                    scalar1=bqk_t[ft_])
                                half_pp = None
                        else:
                            fn(*args)

                    def score_mms(sc, col0, kt):
                        k0 = kt * 128
                        for hh in range(2):
                            r = slice(hh * DH, (hh + 1) * DH)
                            nc.tensor.matmul(
                                sc[:, col0 + hh * 512:col0 + (hh + 1) * 512],
                                lhsT=kTt[r, k0:k0 + 128],
                                rhs=qTt[r, q0:q0 + 512],
                                start=True, stop=True)

                    def pv_mms(pt, col0, kt):
                        for hh in range(2):
                            h = hp * 2 + hh
                            nc.tensor.matmul(
                                ops[hh],
                                lhsT=vx[kt][:, h * (DH + 1):
                                            (h + 1) * (DH + 1)],
                                rhs=pt[:, col0 + hh * 512:
                                        col0 + (hh + 1) * 512],
                                start=(kt == 0), stop=(kt == ST - 1))

                    if EXP15:
                        # 1.5-buffered exp: groups alternate the 4-bank A
                        # pair tile (fd=2048) and the 2-bank B tile
                        # (fd=1024); the next group's scores are emitted
                        # right after this group's exp, so every refill
                        # hides under the other buffer's exp.  PV hh=0 runs
                        # in-loop; hh=1 sweeps afterwards from the buffered
                        # pt tiles (ops bank freed by then).
                        GROUPS = [("A", (0, 1)), ("B", (2,)),
                                  ("A", (3, 4)), ("B", (5,)),
                                  ("A", (6, 7)), ("B", (8,)),
                                  ("A", (9, 10)), ("B", (11,)),
                                  ("A", (12, 13)), ("B", (14,)),
                                  ("B", (15,))]

                        def pv_one(op, hh, pt, col0, kt):
                            h = hp * 2 + hh
                            nc.tensor.matmul(
                                op,
                                lhsT=vx[kt][:, h * (DH + 1):
                                            (h + 1) * (DH + 1)],
                                rhs=pt[:, col0 + hh * 512:
                                        col0 + (hh + 1) * 512],
                                start=(kt == 0), stop=(kt == ST - 1))

                        def group_scores(g):
                            pool_, kts_ = GROUPS[g]
                            p = scpA if pool_ == "A" else scpB
                            sc = p.tile([128, 1024 * len(kts_)], F32,
                                        tag="sc", name=f"sc{hp}_{qc}_g{g}")
                            for j, kt in enumerate(kts_):
                                score_mms(sc, j * 1024, kt)
                            return sc

                        pts = []
                        sc_cur = group_scores(0)
                        for g, (pool_, kts_) in enumerate(GROUPS):
                            pt = ptp.tile([128, 1024 * len(kts_)], BF16,
                                          tag=f"pt{pool_}",
                                          bufs=6 if pool_ == "A" else 8,
                                          name=f"pt{hp}_{qc}_g{g}")
                            nc.scalar.activation(pt, sc_cur, Exp,
                                                 scale=0.125)
                            if g + 1 < len(GROUPS):
                                sc_cur = group_scores(g + 1)
                            for j, kt in enumerate(kts_):
                                pts.append((pt, j * 1024, kt))
                                pv_one(ops[0], 0, pt, j * 1024, kt)
                                emit_filler(kt)
                        # hh=1 sweep from the buffered pt tiles; its PE work
                        # hides under the next loop's exps
                        ops[1] = opp.tile([DH + 1, 512], F32, tag="op",
                                          name=f"op{hp}_{qc}_1")
                        for (pt, c0, kt) in pts:
                            pv_one(ops[1], 1, pt, c0, kt)
                    elif EXP_PAIRS:
                        # one [128,2048] exp per kt-PAIR; the 4-bank pair
                        # psum is single-buffered (all 8 banks committed).
                        # PE-queue order per pair (FIFO!): fillers run while
                        # the ACT exps, the NEXT pair's scores go before the
                        # pv so the ACT restarts after only ~4 matmuls, and
                        # the pv (which only needs pt in SBUF) trails.
                        def pair_scores(kp):
                            sc = scp.tile([128, 2048], F32, tag="sc",
                                          name=f"sc{hp}_{qc}_{kp}")
                            for j in range(2):
                                score_mms(sc, j * 1024, 2 * kp + j)
                            return sc

                        sc_cur = pair_scores(0)
                        for kp in range(ST // 2):
                            pt = ptp.tile([128, 2048], BF16, tag="pt",
                                          name=f"pt{hp}_{qc}_{kp}")
                            nc.scalar.activation(pt, sc_cur, Exp, scale=0.125)
                            emit_filler(2 * kp)
                            emit_filler(2 * kp + 1)
                            if kp + 1 < ST // 2:
                                sc_cur = pair_scores(kp + 1)
                            for j in range(2):
                                pv_mms(pt, j * 1024, 2 * kp + j)
                    else:
                        # lookahead is ILLEGAL in the (qc0, hp0) loop: its
                        # fillers (kv quarters 1-3) produce the k-features
                        # that scores(kt+1) consumes -- hoisting the scores
                        # above the filler would read stale qkT
                        la = SCORE_LOOKAHEAD and not (
                            qc == 0 and hp == 0 and INTERLEAVE_KV)
                        sc_next = None
                        for kt in range(ST):
                            if sc_next is None:
                                sc_next = scp.tile([128, 1024], F32,
                                                   tag="sc",
                                                   name=f"sc{hp}_{qc}_{kt}")
                                score_mms(sc_next, 0, kt)
                            sc = sc_next
                            sc_next = None
                            if la and kt + 1 < ST:
                                sc_next = scp.tile([128, 1024], F32,
                                                   tag="sc",
                                                   name=f"sc{hp}_{qc}_{kt+1}")
                                score_mms(sc_next, 0, kt + 1)
                            pt = ptp.tile([128, 1024], BF16, tag="pt",
                                          name=f"pt{hp}_{qc}_{kt}")
                            nc.scalar.activation(pt, sc, Exp, scale=0.125)
                            pv_mms(pt, 0, kt)
                            emit_filler(kt)
                    # any unemitted filler (shouldn't happen, but be safe)
                    for kt in sorted(filler):
                        f = filler[kt]
                        if callable(f):
                            f()
                        else:
                            fn, args = f
                            fn(*args)
                    for hh in range(2):
                        # one DVE copy drains the psum accumulator (freeing
                        # the bank); normalization runs from SBUF
                        drn = drnp.tile([DH + 1, 512], F32, tag="drn",
                                        name=f"drn{hp}_{qc}_{hh}")
                        nc.vector.tensor_copy(drn, ops[hh])
                        srow = recp.tile([1, 512], F32, tag="srow")
                        nc.vector.tensor_copy(srow, drn[DH:DH + 1, :])
                        rec = recp.tile([1, 512], F32, tag="rec")
                        nc.vector.reciprocal_approx_fast(rec, srow)
                        rb = rbp.tile([DH, 512], F32, tag="rb")
                        nc.gpsimd.partition_broadcast(rb, rec)
                        nc.vector.tensor_mul(
                            out=aoc_cur[hp][hh * DH:(hh + 1) * DH, :],
                            in0=drn[0:DH, :], in1=rb)
                aoc_prev = aoc_cur
            if TAIL_INTERLEAVE == 2:
                # x/w DMAs AFTER the qc=3 loops (so the stale-reading hp=3
                # fillers never wait on them) but BEFORE the epilogue, x
                # first: the next rep's kv-quarter-1 filler (its earliest
                # DMA consumer, ~10us away) waits only on the 8 x tiles.
                inputs_dma_wx(x_first=True)
            # epilogue: out projection for the last chunk.  With EXP_PAIRS the
            # scp pair tile is single-buffered, so run the epilogue through
            # the fill pool (2 bufs) to keep two tiles in flight; otherwise
            # use the (now idle) scp banks as before.
            if EXP_PAIRS or EXP15:
                for sl in range(4):
                    st = 3 * 4 + sl
                    c0 = st * 128
                    lo = sl * 128
                    ot = outp.tile([128, E], F32, tag="ot")
                    for fc in range(2):
                        f0 = fc * 512
                        pp = fill.tile([128, 512], F32, tag="fl",
                                       name=f"ep{sl}_{fc}")
                        for et in range(4):
                            nc.tensor.matmul(
                                pp,
                                lhsT=aoc_prev[et][:, lo:lo + 128],
                                rhs=wpT_t[et][:, f0:f0 + 512],
                                start=(et == 0), stop=(et == 3))
                        nc.vector.tensor_copy(ot[:, f0:f0 + 512], pp)
                    nc.sync.dma_start(out=part[c0:c0 + 128, :], in_=ot)
            else:
                for sl in range(4):
                    st = 3 * 4 + sl
                    c0 = st * 128
                    lo = sl * 128
                    pp = scp.tile([128, 1024], F32, tag="sc", name=f"ep{sl}")
                    for fc in range(2):
                        f0 = fc * 512
                        for et in range(4):
                            nc.tensor.matmul(
                                pp[:, f0:f0 + 512],
                                lhsT=aoc_prev[et][:, lo:lo + 128],
                                rhs=wpT_t[et][:, f0:f0 + 512],
                                start=(et == 0), stop=(et == 3))
                    ot = outp.tile([128, E], F32, tag="ot")
                    nc.vector.tensor_copy(ot, pp)
                    nc.sync.dma_start(out=part[c0:c0 + 128, :], in_=ot)

            # prefetch the next iteration's inputs and precompute its
            # k-quarter-0 / q-projection(qc0) so the next iteration opens
            # directly with score matmuls (software-pipelined loop boundary)
            if TAIL_INTERLEAVE == 1:
                # kv-quarter-0 / q-proj(0) already ran as hp=3 fillers;
                # only the out-projection weights remain (WAR vs epilogue)
                inputs_dma_wp()
            elif TAIL_INTERLEAVE == 2:
                # x/w DMAs already issued before the epilogue; only the
                # out-projection weights remain (WAR vs the epilogue reads)
                inputs_dma_wp()
            else:
                inputs_dma()
                kv_quarter(0)
                for ft in range(4):
                    q_proj_full(0, ft)

    nc.compile()
    return nc


def _get_runner(debug=False, repeats=1, loop_n=0):
    """Build (once) a cached jit'd SPMD runner over the 8 axon cores."""
    key = ("run", debug, repeats, loop_n)
    if key in _CACHE:
        return _CACHE[key]

    import jax
    from jax.experimental.shard_map import shard_map
    from jax.sharding import Mesh, PartitionSpec, NamedSharding
    from concourse.bass2jax import (install_neuronx_cc_hook, _bass_exec_p,
                                    partition_id_tensor)

    nc = _build(debug, repeats, loop_n)
    install_neuronx_cc_hook()

    in_names, out_names, out_avals, zero_outs = [], [], [], []
    partition_name = nc.partition_id_tensor.name if nc.partition_id_tensor else None
    for alloc in nc.m.functions[0].allocations:
        if not isinstance(alloc, mybir.MemoryLocationSet):
            continue
        name = alloc.memorylocations[0].name
        if alloc.kind == "ExternalInput":
            if name != partition_name:
                in_names.append(name)
        elif alloc.kind == "ExternalOutput":
            shape = tuple(alloc.tensor_shape)
            dtype = mybir.dt.np(alloc.dtype)
            out_names.append(name)
            out_avals.append(jax.core.ShapedArray(shape, dtype))
            zero_outs.append(np.zeros(shape, dtype))
    n_params = len(in_names)
    n_outs = len(out_names)
    all_in_names = in_names + out_names
    if partition_name is not None:
        all_in_names.append(partition_name)

    def _body(*args):
        operands = list(args)
        if partition_name is not None:
            operands.append(partition_id_tensor())
        outs = _bass_exec_p.bind(
            *operands,
            out_avals=tuple(out_avals),
            in_names=tuple(all_in_names),
            out_names=tuple(out_names),
            lowering_input_output_aliases=(),
            sim_require_finite=True,
            sim_require_nnan=True,
            nc=nc,
        )
        return tuple(outs)

    devices = jax.devices()[:N_CORES]
    mesh = Mesh(np.asarray(devices), ("core",))
    in_specs = (PartitionSpec("core"),) * (n_params + n_outs)
    out_specs = (PartitionSpec("core"),) * n_outs
    sharded = jax.jit(
        shard_map(_body, mesh=mesh, in_specs=in_specs, out_specs=out_specs,
                  check_rep=False),
        donate_argnums=tuple(range(n_params, n_params + n_outs)),
        keep_unused=True,
    )
    sharded_nodonate = jax.jit(
        shard_map(_body, mesh=mesh, in_specs=in_specs, out_specs=out_specs,
                  check_rep=False),
        keep_unused=True,
    )
    core_sharding = NamedSharding(mesh, PartitionSpec("core"))

    def run(in_maps, timing_iters=0):
        concat_in = [
            np.concatenate([np.asarray(m[name]) for m in in_maps], axis=0)
            for name in in_names
        ]
        concat_zeros = [
            np.zeros((N_CORES * z.shape[0], *z.shape[1:]), z.dtype)
            for z in zero_outs
        ]
        out_arrs = sharded(*concat_in, *concat_zeros)
        results = [
            {name: np.asarray(out_arrs[i]).reshape(N_CORES, *out_avals[i].shape)[c]
             for i, name in enumerate(out_names)}
            for c in range(N_CORES)
        ]
        times = []
        if timing_iters:
            import time
            dev = [jax.device_put(a, core_sharding)
                   for a in concat_in + concat_zeros]
            jax.block_until_ready(dev)
            for _ in range(2):
                jax.block_until_ready(sharded_nodonate(*dev))
            for _ in range(timing_iters):
                t0 = time.perf_counter()
                jax.block_until_ready(sharded_nodonate(*dev))
                times.append(time.perf_counter() - t0)
        return results, times

    _CACHE[key] = run
    return run


def _shard_inputs(x, w_qkv, b_qkv, w_proj):
    x = np.asarray(x, np.float32)
    w = np.asarray(w_qkv, np.float32)
    bq = np.asarray(b_qkv, np.float32)
    wp = np.asarray(w_proj, np.float32)
    if HOST_BF16:
        import ml_dtypes
        in_dt = ml_dtypes.bfloat16
    else:
        in_dt = np.float32
    in_maps = []
    for b in range(B):
        xTb = np.ascontiguousarray(x[b].T.astype(in_dt))        # [E, S]
        for g in range(2):
            r = slice(g * FG, (g + 1) * FG)
            w_slice = np.concatenate([w[0:E][r], w[E:2 * E][r],
                                      w[2 * E:3 * E][r]], axis=0)  # [1536, E]
            in_maps.append({
                "xT": xTb,
                "wT": np.ascontiguousarray(w_slice.T.astype(in_dt)),
                "bqk": np.concatenate([bq[0:E][r], bq[E:2 * E][r]]
                                      ).reshape(2 * FG, 1).astype(np.float32),
                "bv": bq[2 * E:3 * E][r].reshape(1, FG).astype(np.float32),
                "wpT": np.ascontiguousarray(wp[:, r].T.astype(in_dt)),
            })
    return in_maps


def _gather(results, b_proj):
    bp = np.asarray(b_proj, np.float32)
    out = np.empty((B, S, E), np.float32)
    for b in range(B):
        out[b] = results[2 * b]["part"] + results[2 * b + 1]["part"] + bp
    return out


def kernel(x, w_qkv, b_qkv, w_proj, b_proj):
    run = _get_runner()
    in_maps = _shard_inputs(x, w_qkv, b_qkv, w_proj)
    results, _ = run(in_maps)
    return _gather(results, b_proj)


def kernel_timed(x, w_qkv, b_qkv, w_proj, b_proj, iters=5):
    """Like kernel() but also returns per-call device wall times (seconds)."""
    run = _get_runner()
    in_maps = _shard_inputs(x, w_qkv, b_qkv, w_proj)
    results, times = run(in_maps, timing_iters=iters)
    return _gather(results, b_proj), times


def device_time_ns(inputs, loop_n=129, iters=20, rounds=5):
    """Device execution time per kernel invocation (ns), via hardware-loop
    delta: wall(loop_n=N) - wall(loop_n=1) = (N-1) * T_device.  Cancels the
    host/RPC dispatch overhead (~70-140 ms through the axon tunnel), which
    dominates single-call wall time.  Each round pairs a loop_n=1 and a
    loop_n=N measurement under the same network conditions; the median of
    per-round deltas rejects outlier rounds."""
    in_maps = _shard_inputs(inputs["x"], inputs["w_qkv"], inputs["b_qkv"],
                            inputs["w_proj"])
    r1 = _get_runner(loop_n=0)
    rN = _get_runner(loop_n=loop_n)
    deltas = []
    for _ in range(rounds):
        _, t1 = r1(in_maps, timing_iters=iters)
        _, tN = rN(in_maps, timing_iters=iters)
        deltas.append((min(tN) - min(t1)) / (loop_n - 1) * 1e9)
    deltas.sort()
    # lower-median: drift only ever inflates a round, never deflates it
    return deltas[(len(deltas) - 1) // 2]



# revision 43
# speedup vs baseline: 1.1817x; 1.0026x over previous
"""Multi-head attention (B=4, S=2048, E=1024, H=16, Dh=64) on 8 TRN2 NeuronCores.

Sharding: data-parallel over batch (4) x tensor-parallel over head-groups (2).
Core (b, g) computes heads g*8 .. g*8+7 of batch b end-to-end: qkv projection,
attention, and the output-projection partial sum over its 512 attention-output
features.  The host sums the two per-batch partials and adds b_proj.

v4 (vs the fp32r baseline):
  - all matmul operands bf16 (fp32 PSUM accumulate); inputs cast on device
    via gpsimd casting DMAs.  exp writes bf16.  ACT runs exp only.
  - x loaded once (resident bf16 [8][128, 2048]); all inputs arrive as 20
    full-tile casting DMAs per iteration (q|k|v weight slices merged into one
    [128,1536] tile per e-tile), prefetched at the previous iteration's tail.
  - phase A (k/v projection) quarter-interleaved into the first head-pair's
    attention k-loop, so the scalar engine starts exp ~25us into the kernel
    instead of ~90us.
  - q-proj / out-proj PE work emitted as small pieces inside the attention
    k-loop (instead of ahead of each head-pair) to keep ACT fed.
  - attention-output psum is drained to SBUF with one DVE copy per head so
    the psum bank frees ~2us earlier; normalization runs from SBUF.

v8 session notes (all numbers hw-measured via loop-delta microbenches):
  - the kernel is ACT-bound: exp [128,1024] psum->sbuf costs 1403ns back to
    back (785ns fixed overhead + 0.604ns/elem), so 256 exps = 359us of the
    ~405us kernel.  bf16 matmuls N=512 stream at 110ns (2 cols/cycle), so
    the PE (1632 MMs) is only ~180us busy -- NOT the bottleneck.
  - exp free-dim is capped at 1024 by PSUM: scp 2x[128,1024] (4 banks) +
    ops 2x[65,512] (2) + fill 2x[128,512] (2) = all 8 banks.
  - things measured SLOWER and reverted: Schraudolph-on-DVE exp offload of
    4/16 kt tiles (444us plain / 420us with high_priority+retuned kts; the
    cross-engine coupling on the sc double-buffer eats the ACT savings);
    interleaving the rep tail into qc=3 (422us); fp8 PV fails precision
    (3.7e-2 > 2e-2: softmax concentrates on large p, errors don't average).
  - SCORE_LOOKAHEAD (scores one kt ahead of pv) is neutral (405us) -- the
    Tile scheduler already reorders around the exp->pv stall.
"""

import numpy as np
from contextlib import ExitStack

import concourse.bacc as bacc
import concourse.bass as bass
import concourse.tile as tile
import concourse.mybir as mybir

B, S, E, H, DH = 4, 2048, 1024, 16, 64
N_CORES = 8
FG = 512          # features per head-group (8 heads x 64)
HG = 8            # heads per core
ET = E // 128     # 8 e-tiles (qkv contraction)
ST = S // 128     # 16 s-tiles
F32 = mybir.dt.float32
F32R = mybir.dt.float32r
BF16 = mybir.dt.bfloat16

_CACHE: dict = {}
INTERLEAVE_KV = True
QPROJ_HALVES = True
ROTATE_DMA = True
# v7: software-pipeline the rep boundary INTO the last head-pair's attention
# sweep: next rep's kv-quarter-0 + q-proj(qc0) run as qc=3/hp=3 fillers.
# Mode 1: the w/x casting DMAs also move to qc=3's head so the fillers read
#   fresh data.  (measured: -17us SLOWER on hw -- the 20 casting DMAs
#   contend on the Pool queue during qc=3 and the wpT WAR serializes)
# Mode 2: fillers read the CURRENT resident w/x tiles (a WAR against the
#   tail DMAs, which never stalls the readers).  The weights are constant
#   and x is rep-invariant in the timing loop, so the values are identical.
#   Measured 443us with tail DMAs (next rep's kv-quarter-1 stalls on the
#   20-DMA Pool chain) and 444us with pre-epilogue x-first DMAs -- the
#   residual regression is the 13-filler concentration in hp=3 (DVE drain
#   backlog delays the normalization -> epilogue chain).  Every tail-
#   interleave variant measured SLOWER than the plain 30us serial tail;
#   keep False.
TAIL_INTERLEAVE = False
# v8: emit the scores matmuls one kt ahead of the pv matmuls.  The PE queue
# is strict FIFO: with program order [sc(kt), pv(kt), sc(kt+1), ...] the pv
# stalls the queue until exp(kt) completes, which delays sc(kt+1) and hence
# exp(kt+1) -- the ACT engine (the bottleneck at 1403ns per exp) sees a
# bubble every iteration.  Order [sc(kt+1), pv(kt), ...] keeps the next
# exp's input ready before the stall.  (measured neutral on hw)
SCORE_LOOKAHEAD = True
# v11: 1.5-buffered exp -- alternate a 4-bank pair tile A (fd=2048) with a
# 2-bank single tile B (fd=1024): 5 pair-exps + 6 single-exps per loop =
# 18.5us of ACT vs 16x1403=22.4us, and every refill hides under the OTHER
# buffer's exp (unlike v9's single buffer).  Bank budget A4+B2+opp1+fill1=8
# requires: PV split into sequential per-head sweeps (pt tiles buffered in
# SBUF, one [65,512] accumulator bank live at a time) and fill bufs=1.
# MEASURED 476811ns vs the 409585 checkpoint (numerics exact) -- like v9,
# the hidden exp->refill->exp turnaround cost (~1-2us per buffer handoff)
# swamps the 63us ACT-overhead saving.  Keep OFF.
EXP15 = False
# v12: pre-cast x / w_qkv / w_proj to bf16 on the HOST (the kernel casts
# them to bf16 anyway; _shard_inputs already does host-side transposes).
# Halves the per-rep DMA read traffic (16.6MB fp32 -> 8.3MB bf16), which
# shortens the serial DMA chain exposed at the rep boundary.
HOST_BF16 = True
# v13: with bf16 inputs the DMAs no longer cast, so they are no longer
# restricted to the gpsimd queue.  Route the x loads over the sync (HWDGE)
# queue so the two ~11us chains run in parallel, halving the serial DMA
# wall at the rep tail (the tail kv/q-proj compute waits on all of them).
DMA_SPLIT = True
# v9: halve the ACT instruction count by exping kt-PAIRS: one [128,2048]
# activation (785ns overhead + 0.604ns/elem ~= 2022ns) instead of two
# [128,1024] ones (2x1403ns).  The pair's scores psum tile is 4 banks --
# with opp(2) + fill(2) that is all 8, so the pair buffer is SINGLE-
# buffered: scores(pair p+1) wait for exp(p).  Predicted ~342us; MEASURED
# 497us on hw -- the exposed refill serialization costs ~1.9us/pair, far
# more than the sem+4-matmul model.  Keep OFF.
EXP_PAIRS = False


def _build(debug=False, repeats=1, loop_n=0):
    nc = bacc.Bacc("TRN2", target_bir_lowering=False, debug=False,
                   num_devices=N_CORES)
    IN_DT = BF16 if HOST_BF16 else F32
    xT = nc.dram_tensor("xT", [E, S], IN_DT, kind="ExternalInput").ap()
    wT = nc.dram_tensor("wT", [E, 3 * FG], IN_DT, kind="ExternalInput").ap()
    bqk = nc.dram_tensor("bqk", [2 * FG, 1], F32, kind="ExternalInput").ap()
    bv = nc.dram_tensor("bv", [1, FG], F32, kind="ExternalInput").ap()
    wpT = nc.dram_tensor("wpT", [FG, E], IN_DT, kind="ExternalInput").ap()
    part = nc.dram_tensor("part", [S, E], F32, kind="ExternalOutput").ap()

    Exp = mybir.ActivationFunctionType.Exp

    with tile.TileContext(nc) as tc, ExitStack() as ctx:
        # ---- long-lived tiles ----
        pers = ctx.enter_context(tc.tile_pool(name="pers", bufs=1))

        vb = pers.tile([128, FG], F32, tag="vb")
        nc.sync.dma_start(out=vb, in_=bv.partition_broadcast(128))
        bqk_t = []
        for ft in range(8):
            t = pers.tile([128, 1], F32, tag=f"bqk{ft}", name=f"bqk{ft}")
            nc.sync.dma_start(out=t, in_=bqk[ft * 128:(ft + 1) * 128, :])
            bqk_t.append(t)
        # q/k feature-major tiles: ft 0..3 = q features, 4..7 = k features
        qkT = [pers.tile([128, S], BF16, tag=f"qk{ft}", name=f"qk{ft}")
               for ft in range(8)]
        # v_ext: [s-tile, (8 heads x (64 v cols + ones col))]
        ones = pers.tile([128, HG], F32, tag="ones")
        nc.vector.memset(ones, 1.0)
        vx = []
        for st in range(ST):
            t = pers.tile([128, HG * (DH + 1)], BF16, tag=f"vx{st}",
                          name=f"vx{st}")
            nc.vector.tensor_copy(
                t.rearrange("p (h c) -> p h c", c=DH + 1)[:, :, DH], ones)
            vx.append(t)
        # weights resident in bf16 (cast during DMA on the gpsimd queue);
        # q|k|v slices merged into one tile per e-tile => one DMA each
        wqkv_t = [pers.tile([128, 3 * FG], BF16, tag=f"wqkv{e}",
                            name=f"wqkv{e}") for e in range(ET)]
        wpT_t = [pers.tile([128, E], BF16, tag=f"wp{et}", name=f"wp{et}")
                 for et in range(4)]
        # x resident in bf16, reloaded once per rep
        xb = [pers.tile([128, S], BF16, tag=f"xb{e}", name=f"xb{e}")
              for e in range(ET)]


        fill = ctx.enter_context(tc.tile_pool(name="fill",
                                              bufs=1 if EXP15 else 2,
                                              space="PSUM"))
        # EXP_PAIRS: one [128,2048] pair tile = 4 banks; with opp(2)+fill(2)
        # that is all 8 PSUM banks, so the pair buffer cannot double-buffer
        if EXP15:
            scpA = ctx.enter_context(tc.tile_pool(name="scpA", bufs=1,
                                                  space="PSUM"))
            scpB = ctx.enter_context(tc.tile_pool(name="scpB", bufs=1,
                                                  space="PSUM"))
            scp = None
        else:
            scp = ctx.enter_context(tc.tile_pool(name="scp",
                                                 bufs=1 if EXP_PAIRS else 2,
                                                 space="PSUM"))
        opp = ctx.enter_context(tc.tile_pool(name="opp",
                                             bufs=1 if EXP15 else 2,
                                             space="PSUM"))
        ptp = ctx.enter_context(tc.tile_pool(name="ptp", bufs=4))
        aocp = ctx.enter_context(tc.tile_pool(name="aocp", bufs=2))
        drnp = ctx.enter_context(tc.tile_pool(name="drnp", bufs=3))
        recp = ctx.enter_context(tc.tile_pool(name="recp", bufs=2))
        outp = ctx.enter_context(tc.tile_pool(name="outp", bufs=2))
        rbp = ctx.enter_context(tc.tile_pool(name="rbp", bufs=2))

        # ---------- emission helpers ----------
        # fp32 inputs MUST load via gpsimd (casting DMA); bf16 inputs may
        # split across queues for parallel chains
        x_eng = nc.sync if (HOST_BF16 and DMA_SPLIT) else nc.gpsimd

        def inputs_dma_wx(x_first=False):
            order = ["x", "w"] if x_first else ["w", "x"]
            for which in order:
                for e in range(ET):
                    if which == "w":
                        nc.gpsimd.dma_start(
                            out=wqkv_t[e], in_=wT[e * 128:(e + 1) * 128, :])
                    else:
                        x_eng.dma_start(
                            out=xb[e], in_=xT[e * 128:(e + 1) * 128, :])

        def inputs_dma_wp():
            for et in range(4):
                nc.gpsimd.dma_start(
                    out=wpT_t[et], in_=wpT[et * 128:(et + 1) * 128, :])

        def inputs_dma():
            inputs_dma_wx()
            inputs_dma_wp()

        def kf_piece(sq, kf):
            s0 = sq * 512
            ft = 4 + kf
            pp = fill.tile([128, 512], F32, tag="fl",
                           name=f"kp{sq}_{kf}")
            for e in range(ET):
                nc.tensor.matmul(
                    pp, lhsT=wqkv_t[e][:, FG + kf * 128:FG + (kf + 1) * 128],
                    rhs=xb[e][:, s0:s0 + 512], start=(e == 0),
                    stop=(e == ET - 1))
            nc.vector.tensor_scalar_add(
                out=qkT[ft][:, s0:s0 + 512], in0=pp,
                scalar1=bqk_t[ft])

        def v_piece(st):
            sq, sl = st // 4, st % 4
            s0 = sq * 512
            pp = fill.tile([128, FG], F32, tag="fl",
                           name=f"vp{st}")
            for e in range(ET):
                nc.tensor.matmul(
                    pp, lhsT=xb[e][:, s0 + sl * 128:s0 + (sl + 1) * 128],
                    rhs=wqkv_t[e][:, 2 * FG:3 * FG],
                    start=(e == 0), stop=(e == ET - 1))
            nc.vector.tensor_add(
                out=vx[st].rearrange("p (h c) -> p h c",
                                     c=DH + 1)[:, :, 0:DH],
                in0=pp.rearrange("p (h c) -> p h c", c=DH),
                in1=vb.rearrange("p (h c) -> p h c", c=DH))

        def kv_quarter(sq):
            for kf in range(4):
                kf_piece(sq, kf)
            for sl in range(4):
                v_piece(sq * 4 + sl)

        def q_proj_half(qc, ft, half):
            """4 of the 8 contraction matmuls for one q feature tile."""
            s0 = qc * 512
            pp = fill.tile([128, 512], F32, tag="fl",
                           name=f"qp{qc}_{ft}_{half}")
            es = range(0, 4) if half == 0 else range(4, ET)
            for e in es:
                nc.tensor.matmul(
                    pp, lhsT=wqkv_t[e][:, ft * 128:(ft + 1) * 128],
                    rhs=xb[e][:, s0:s0 + 512],
                    start=(e == es.start), stop=False)
            if half == 0:
                return pp
            nc.vector.tensor_scalar_add(
                out=qkT[ft][:, s0:s0 + 512], in0=pp, scalar1=bqk_t[ft])
            return None

        def q_proj_full(qc, ft):
            s0 = qc * 512
            pp = fill.tile([128, 512], F32, tag="fl",
                           name=f"qpf{qc}_{ft}")
            for e in range(ET):
                nc.tensor.matmul(
                    pp, lhsT=wqkv_t[e][:, ft * 128:(ft + 1) * 128],
                    rhs=xb[e][:, s0:s0 + 512],
                    start=(e == 0), stop=(e == ET - 1))
            nc.vector.tensor_scalar_add(
                out=qkT[ft][:, s0:s0 + 512], in0=pp, scalar1=bqk_t[ft])

        def out_proj(qc_prev, sl, aoc_prev):
            st = qc_prev * 4 + sl
            c0 = st * 128
            lo = sl * 128
            ot = outp.tile([128, E], F32, tag="ot")
            for fc in range(2):
                f0 = fc * 512
                pp = fill.tile([128, 512], F32, tag="fl",
                               name=f"pj{st}_{fc}")
                for et in range(4):
                    nc.tensor.matmul(
                        pp, lhsT=aoc_prev[et][:, lo:lo + 128],
                        rhs=wpT_t[et][:, f0:f0 + 512],
                        start=(et == 0), stop=(et == 3))
                nc.vector.tensor_copy(ot[:, f0:f0 + 512], pp)
            nc.sync.dma_start(out=part[c0:c0 + 128, :], in_=ot)

        # prologue: first iteration's inputs + its k-quarter-0 and qc0
        # q-projection, emitted once ahead of the loop (each iteration's
        # tail then produces these for the next iteration)
        inputs_dma()
        kv_quarter(0)
        for ft in range(4):
            q_proj_full(0, ft)

        import contextlib
        rep_ctx = (tc.For_i(0, loop_n, 1, name="bench")
                   if loop_n else contextlib.nullcontext())
        with rep_ctx:
          for _rep in range(repeats):

            # ---------- emission ----------
            if not INTERLEAVE_KV:
                for sq in range(1, 4):
                    kv_quarter(sq)

            aoc_prev = None
            for qc in range(4):
                q0 = qc * 512
                if qc == 3 and TAIL_INTERLEAVE == 1:
                    # next rep's w_qkv/x casting DMAs: started here so the
                    # tail compute (interleaved into hp=3 below) has its
                    # inputs ~20us before it needs them.  wpT DMAs stay at
                    # the rep tail: the qc=3 out-projection still reads wpT_t.
                    inputs_dma_wx()
                aoc_cur = [aocp.tile([128, 512], BF16, tag=f"aoc{et}",
                                     name=f"aoc{qc}_{et}")
                           for et in range(4)]
                for hp in range(4):
                    # PE filler pieces to emit inside the kt loop, keyed by kt
                    filler = {}
                    if qc == 0 and hp == 0:
                        # overlap remaining k/v quarters with the first
                        # head-pair's attention sweep.  Under EXP15 the
                        # group loop emits scores one GROUP ahead, so each
                        # kv quarter must precede the scores of its first
                        # kt by two groups: kt 1/5/9 (vs 3/7/11).
                        if INTERLEAVE_KV:
                            ks = (1, 5, 9) if EXP15 else (3, 7, 11)
                            filler[ks[0]] = lambda: kv_quarter(1)
                            filler[ks[1]] = lambda: kv_quarter(2)
                            filler[ks[2]] = lambda: kv_quarter(3)
                    elif qc == 3 and hp == 3 and TAIL_INTERLEAVE:
                        # software-pipelined rep boundary: next rep's
                        # kv-quarter-0 and q-projection(qc0) interleave into
                        # the last head-pair's attention sweep (slots chosen
                        # WAR-safe: kf3 after this hp's scores pass kt=3,
                        # v(st) after PV(kt=st))
                        filler[1] = (kf_piece, (0, 0))
                        filler[2] = (out_proj, (2, 3, aoc_prev))
                        filler[3] = (kf_piece, (0, 1))
                        filler[4] = (kf_piece, (0, 2))
                        filler[5] = (kf_piece, (0, 3))
                        filler[6] = (v_piece, (0,))
                        filler[7] = (q_proj_full, (0, 0))
                        filler[8] = (v_piece, (1,))
                        filler[9] = (q_proj_full, (0, 1))
                        filler[10] = (v_piece, (2,))
                        filler[11] = (q_proj_full, (0, 2))
                        filler[12] = (v_piece, (3,))
                        filler[13] = (q_proj_full, (0, 3))
                    else:
                        pieces = []
                        if qc < 3:
                            if qc == 0:
                                # 4 q feature tiles over head-pairs 1..3
                                fts = {1: [0], 2: [1], 3: [2, 3]}[hp]
                                for ft in fts:
                                    if QPROJ_HALVES:
                                        pieces.append(
                                            (q_proj_half, (qc + 1, ft, 0)))
                                        pieces.append(
                                            (q_proj_half, (qc + 1, ft, 1)))
                                    else:
                                        pieces.append(
                                            (q_proj_full, (qc + 1, ft)))
                            elif QPROJ_HALVES:
                                pieces.append((q_proj_half, (qc + 1, hp, 0)))
                                pieces.append((q_proj_half, (qc + 1, hp, 1)))
                            else:
                                pieces.append((q_proj_full, (qc + 1, hp)))
                        if qc > 0:
                            pieces.append((out_proj, (qc - 1, hp, aoc_prev)))
                        slots = [2, 5, 8, 11][:len(pieces)]
                        for s, p in zip(slots, pieces):
                            filler[s] = (p[0], p[1])

                    qTt, kTt = qkT[hp], qkT[4 + hp]
                    if EXP15:
                        # sequential per-head accumulators (1 bank live)
                        ops = [None, None]
                        ops[0] = opp.tile([DH + 1, 512], F32, tag="op",
                                          name=f"op{hp}_{qc}_0")
                    else:
                        ops = []
                        for hh in range(2):
                            op = opp.tile([DH + 1, 512], F32, tag="op",
                                          name=f"op{hp}_{qc}_{hh}")
                            ops.append(op)
                    half_pp = None

                    def emit_filler(kt):
                        nonlocal half_pp
                        f = filler.pop(kt, None)
                        if f is None:
                            return
                        if callable(f):
                            f()
                            return
                        fn, args = f
                        if fn is q_proj_half:
                            if args[2] == 0:
                                half_pp = fn(*args)
                            else:
                                # second half continues on half_pp
                                qc_, ft_, _ = args
                                s0_ = qc_ * 512
                                for e in range(4, ET):
                                    nc.tensor.matmul(
                                        half_pp,
                                        lhsT=wqkv_t[e][:, ft_ * 128:
                                                       (ft_ + 1) * 128],
                                        rhs=xb[e][:, s0_:s0_ + 512],
                                        start=False,
                                        stop=(e == ET - 1))
                                nc.vector.tensor_scalar_add(
                                    out=qkT[ft_][:, s0_:s0_ + 512],
                                    in0=half_pp,
                                    scalar1=bqk_t[ft_])
                                half_pp = None
                        else:
                            fn(*args)

                    def score_mms(sc, col0, kt):
                        k0 = kt * 128
                        for hh in range(2):
                            r = slice(hh * DH, (hh + 1) * DH)
                            nc.tensor.matmul(
                                sc[:, col0 + hh * 512:col0 + (hh + 1) * 512],
                                lhsT=kTt[r, k0:k0 + 128],
                                rhs=qTt[r, q0:q0 + 512],
                                start=True, stop=True)

                    def pv_mms(pt, col0, kt):
                        for hh in range(2):
                            h = hp * 2 + hh
                            nc.tensor.matmul(
                                ops[hh],
                                lhsT=vx[kt][:, h * (DH + 1):
                                            (h + 1) * (DH + 1)],
                                rhs=pt[:, col0 + hh * 512:
                                        col0 + (hh + 1) * 512],
                                start=(kt == 0), stop=(kt == ST - 1))

                    if EXP15:
                        # 1.5-buffered exp: groups alternate the 4-bank A
                        # pair tile (fd=2048) and the 2-bank B tile
                        # (fd=1024); the next group's scores are emitted
                        # right after this group's exp, so every refill
                        # hides under the other buffer's exp.  PV hh=0 runs
                        # in-loop; hh=1 sweeps afterwards from the buffered
                        # pt tiles (ops bank freed by then).
                        GROUPS = [("A", (0, 1)), ("B", (2,)),
                                  ("A", (3, 4)), ("B", (5,)),
                                  ("A", (6, 7)), ("B", (8,)),
                                  ("A", (9, 10)), ("B", (11,)),
                                  ("A", (12, 13)), ("B", (14,)),
                                  ("B", (15,))]

                        def pv_one(op, hh, pt, col0, kt):
                            h = hp * 2 + hh
                            nc.tensor.matmul(
                                op,
                                lhsT=vx[kt][:, h * (DH + 1):
                                            (h + 1) * (DH + 1)],
                                rhs=pt[:, col0 + hh * 512:
                                        col0 + (hh + 1) * 512],
                                start=(kt == 0), stop=(kt == ST - 1))

                        def group_scores(g):
                            pool_, kts_ = GROUPS[g]
                            p = scpA if pool_ == "A" else scpB
                            sc = p.tile([128, 1024 * len(kts_)], F32,
                                        tag="sc", name=f"sc{hp}_{qc}_g{g}")
                            for j, kt in enumerate(kts_):
                                score_mms(sc, j * 1024, kt)
                            return sc

                        pts = []
                        sc_cur = group_scores(0)
                        for g, (pool_, kts_) in enumerate(GROUPS):
                            pt = ptp.tile([128, 1024 * len(kts_)], BF16,
                                          tag=f"pt{pool_}",
                                          bufs=6 if pool_ == "A" else 8,
                                          name=f"pt{hp}_{qc}_g{g}")
                            nc.scalar.activation(pt, sc_cur, Exp,
                                                 scale=0.125)
                            if g + 1 < len(GROUPS):
                                sc_cur = group_scores(g + 1)
                            for j, kt in enumerate(kts_):
                                pts.append((pt, j * 1024, kt))
                                pv_one(ops[0], 0, pt, j * 1024, kt)
                                emit_filler(kt)
                        # hh=1 sweep from the buffered pt tiles; its PE work
                        # hides under the next loop's exps
                        ops[1] = opp.tile([DH + 1, 512], F32, tag="op",
                                          name=f"op{hp}_{qc}_1")
                        for (pt, c0, kt) in pts:
                            pv_one(ops[1], 1, pt, c0, kt)
                    elif EXP_PAIRS:
                        # one [128,2048] exp per kt-PAIR; the 4-bank pair
                        # psum is single-buffered (all 8 banks committed).
                        # PE-queue order per pair (FIFO!): fillers run while
                        # the ACT exps, the NEXT pair's scores go before the
                        # pv so the ACT restarts after only ~4 matmuls, and
                        # the pv (which only needs pt in SBUF) trails.
                        def pair_scores(kp):
                            sc = scp.tile([128, 2048], F32, tag="sc",
                                          name=f"sc{hp}_{qc}_{kp}")
                            for j in range(2):
                                score_mms(sc, j * 1024, 2 * kp + j)
                            return sc

                        sc_cur = pair_scores(0)
                        for kp in range(ST // 2):
                            pt = ptp.tile([128, 2048], BF16, tag="pt",
                                          name=f"pt{hp}_{qc}_{kp}")
                            nc.scalar.activation(pt, sc_cur, Exp, scale=0.125)
                            emit_filler(2 * kp)
                            emit_filler(2 * kp + 1)
                            if kp + 1 < ST // 2:
                                sc_cur = pair_scores(kp + 1)
                            for j in range(2):
                                pv_mms(pt, j * 1024, 2 * kp + j)
                    else:
                        # lookahead is ILLEGAL in the (qc0, hp0) loop: its
                        # fillers (kv quarters 1-3) produce the k-features
                        # that scores(kt+1) consumes -- hoisting the scores
                        # above the filler would read stale qkT
                        la = SCORE_LOOKAHEAD and not (
                            qc == 0 and hp == 0 and INTERLEAVE_KV)
                        sc_next = None
                        for kt in range(ST):
                            if sc_next is None:
                                sc_next = scp.tile([128, 1024], F32,
                                                   tag="sc",
                                                   name=f"sc{hp}_{qc}_{kt}")
                                score_mms(sc_next, 0, kt)
                            sc = sc_next
                            sc_next = None
                            if la and kt + 1 < ST:
                                sc_next = scp.tile([128, 1024], F32,
                                                   tag="sc",
                                                   name=f"sc{hp}_{qc}_{kt+1}")
                                score_mms(sc_next, 0, kt + 1)
                            pt = ptp.tile([128, 1024], BF16, tag="pt",
                                          name=f"pt{hp}_{qc}_{kt}")
                            nc.scalar.activation(pt, sc, Exp, scale=0.125)
                            pv_mms(pt, 0, kt)
                            emit_filler(kt)
                    # any unemitted filler (shouldn't happen, but be safe)
                    for kt in sorted(filler):
                        f = filler[kt]
                        if callable(f):
                            f()
                        else:
                            fn, args = f
                            fn(*args)
                    for hh in range(2):
                        # one DVE copy drains the psum accumulator (freeing
                        # the bank); normalization runs from SBUF
                        drn = drnp.tile([DH + 1, 512], F32, tag="drn",
                                        name=f"drn{hp}_{qc}_{hh}")
                        nc.vector.tensor_copy(drn, ops[hh])
                        srow = recp.tile([1, 512], F32, tag="srow")
                        nc.vector.tensor_copy(srow, drn[DH:DH + 1, :])
                        rec = recp.tile([1, 512], F32, tag="rec")
                        nc.vector.reciprocal_approx_fast(rec, srow)
                        rb = rbp.tile([DH, 512], F32, tag="rb")
                        nc.gpsimd.partition_broadcast(rb, rec)
                        nc.vector.tensor_mul(
                            out=aoc_cur[hp][hh * DH:(hh + 1) * DH, :],
                            in0=drn[0:DH, :], in1=rb)
                aoc_prev = aoc_cur
            if TAIL_INTERLEAVE == 2:
                # x/w DMAs AFTER the qc=3 loops (so the stale-reading hp=3
                # fillers never wait on them) but BEFORE the epilogue, x
                # first: the next rep's kv-quarter-1 filler (its earliest
                # DMA consumer, ~10us away) waits only on the 8 x tiles.
                inputs_dma_wx(x_first=True)
            # epilogue: out projection for the last chunk.  With EXP_PAIRS the
            # scp pair tile is single-buffered, so run the epilogue through
            # the fill pool (2 bufs) to keep two tiles in flight; otherwise
            # use the (now idle) scp banks as before.
            if EXP_PAIRS or EXP15:
                for sl in range(4):
                    st = 3 * 4 + sl
                    c0 = st * 128
                    lo = sl * 128
                    ot = outp.tile([128, E], F32, tag="ot")
                    for fc in range(2):
                        f0 = fc * 512
                        pp = fill.tile([128, 512], F32, tag="fl",
                                       name=f"ep{sl}_{fc}")
                        for et in range(4):
                            nc.tensor.matmul(
                                pp,
                                lhsT=aoc_prev[et][:, lo:lo + 128],
                                rhs=wpT_t[et][:, f0:f0 + 512],
                                start=(et == 0), stop=(et == 3))
                        nc.vector.tensor_copy(ot[:, f0:f0 + 512], pp)
                    nc.sync.dma_start(out=part[c0:c0 + 128, :], in_=ot)
            else:
                for sl in range(4):
                    st = 3 * 4 + sl
                    c0 = st * 128
                    lo = sl * 128
                    pp = scp.tile([128, 1024], F32, tag="sc", name=f"ep{sl}")
                    for fc in range(2):
                        f0 = fc * 512
                        for et in range(4):
                            nc.tensor.matmul(
                                pp[:, f0:f0 + 512],
                                lhsT=aoc_prev[et][:, lo:lo + 128],
                                rhs=wpT_t[et][:, f0:f0 + 512],
                                start=(et == 0), stop=(et == 3))
                    ot = outp.tile([128, E], F32, tag="ot")
                    nc.vector.tensor_copy(ot, pp)
                    nc.sync.dma_start(out=part[c0:c0 + 128, :], in_=ot)

            # prefetch the next iteration's inputs and precompute its
            # k-quarter-0 / q-projection(qc0) so the next iteration opens
            # directly with score matmuls (software-pipelined loop boundary)
            if TAIL_INTERLEAVE == 1:
                # kv-quarter-0 / q-proj(0) already ran as hp=3 fillers;
                # only the out-projection weights remain (WAR vs epilogue)
                inputs_dma_wp()
            elif TAIL_INTERLEAVE == 2:
                # x/w DMAs already issued before the epilogue; only the
                # out-projection weights remain (WAR vs the epilogue reads)
                inputs_dma_wp()
            else:
                inputs_dma()
                kv_quarter(0)
                for ft in range(4):
                    q_proj_full(0, ft)

    nc.compile()
    return nc


def _get_runner(debug=False, repeats=1, loop_n=0):
    """Build (once) a cached jit'd SPMD runner over the 8 axon cores."""
    key = ("run", debug, repeats, loop_n)
    if key in _CACHE:
        return _CACHE[key]

    import jax
    from jax.experimental.shard_map import shard_map
    from jax.sharding import Mesh, PartitionSpec, NamedSharding
    from concourse.bass2jax import (install_neuronx_cc_hook, _bass_exec_p,
                                    partition_id_tensor)

    nc = _build(debug, repeats, loop_n)
    install_neuronx_cc_hook()

    in_names, out_names, out_avals, zero_outs = [], [], [], []
    partition_name = nc.partition_id_tensor.name if nc.partition_id_tensor else None
    for alloc in nc.m.functions[0].allocations:
        if not isinstance(alloc, mybir.MemoryLocationSet):
            continue
        name = alloc.memorylocations[0].name
        if alloc.kind == "ExternalInput":
            if name != partition_name:
                in_names.append(name)
        elif alloc.kind == "ExternalOutput":
            shape = tuple(alloc.tensor_shape)
            dtype = mybir.dt.np(alloc.dtype)
            out_names.append(name)
            out_avals.append(jax.core.ShapedArray(shape, dtype))
            zero_outs.append(np.zeros(shape, dtype))
    n_params = len(in_names)
    n_outs = len(out_names)
    all_in_names = in_names + out_names
    if partition_name is not None:
        all_in_names.append(partition_name)

    def _body(*args):
        operands = list(args)
        if partition_name is not None:
            operands.append(partition_id_tensor())
        outs = _bass_exec_p.bind(
            *operands,
            out_avals=tuple(out_avals),
            in_names=tuple(all_in_names),
            out_names=tuple(out_names),
            lowering_input_output_aliases=(),
            sim_require_finite=True,
            sim_require_nnan=True,
            nc=nc,
        )
        return tuple(outs)

    devices = jax.devices()[:N_CORES]
    mesh = Mesh(np.asarray(devices), ("core",))
    in_specs = (PartitionSpec("core"),) * (n_params + n_outs)
    out_specs = (PartitionSpec("core"),) * n_outs
    sharded = jax.jit(
        shard_map(_body, mesh=mesh, in_specs=in_specs, out_specs=out_specs,
                  check_rep=False),
        donate_argnums=tuple(range(n_params, n_params + n_outs)),
        keep_unused=True,
    )
    sharded_nodonate = jax.jit(
        shard_map(_body, mesh=mesh, in_specs=in_specs, out_specs=out_specs,
                  check_rep=False),
        keep_unused=True,
    )
    core_sharding = NamedSharding(mesh, PartitionSpec("core"))

    def run(in_maps, timing_iters=0):
        concat_in = [
            np.concatenate([np.asarray(m[name]) for m in in_maps], axis=0)
            for name in in_names
        ]
        concat_zeros = [
            np.zeros((N_CORES * z.shape[0], *z.shape[1:]), z.dtype)
            for z in zero_outs
        ]
        out_arrs = sharded(*concat_in, *concat_zeros)
        results = [
            {name: np.asarray(out_arrs[i]).reshape(N_CORES, *out_avals[i].shape)[c]
             for i, name in enumerate(out_names)}
            for c in range(N_CORES)
        ]
        times = []
        if timing_iters:
            import time
            dev = [jax.device_put(a, core_sharding)
                   for a in concat_in + concat_zeros]
            jax.block_until_ready(dev)
            for _ in range(2):
                jax.block_until_ready(sharded_nodonate(*dev))
            for _ in range(timing_iters):
                t0 = time.perf_counter()
                jax.block_until_ready(sharded_nodonate(*dev))
                times.append(time.perf_counter() - t0)
        return results, times

    _CACHE[key] = run
    return run


def _shard_inputs(x, w_qkv, b_qkv, w_proj):
    x = np.asarray(x, np.float32)
    w = np.asarray(w_qkv, np.float32)
    bq = np.asarray(b_qkv, np.float32)
    wp = np.asarray(w_proj, np.float32)
    if HOST_BF16:
        import ml_dtypes
        in_dt = ml_dtypes.bfloat16
    else:
        in_dt = np.float32
    in_maps = []
    for b in range(B):
        xTb = np.ascontiguousarray(x[b].T.astype(in_dt))        # [E, S]
        for g in range(2):
            r = slice(g * FG, (g + 1) * FG)
            w_slice = np.concatenate([w[0:E][r], w[E:2 * E][r],
                                      w[2 * E:3 * E][r]], axis=0)  # [1536, E]
            in_maps.append({
                "xT": xTb,
                "wT": np.ascontiguousarray(w_slice.T.astype(in_dt)),
                "bqk": np.concatenate([bq[0:E][r], bq[E:2 * E][r]]
                                      ).reshape(2 * FG, 1).astype(np.float32),
                "bv": bq[2 * E:3 * E][r].reshape(1, FG).astype(np.float32),
                "wpT": np.ascontiguousarray(wp[:, r].T.astype(in_dt)),
            })
    return in_maps


def _gather(results, b_proj):
    bp = np.asarray(b_proj, np.float32)
    out = np.empty((B, S, E), np.float32)
    for b in range(B):
        out[b] = results[2 * b]["part"] + results[2 * b + 1]["part"] + bp
    return out


def kernel(x, w_qkv, b_qkv, w_proj, b_proj):
    run = _get_runner()
    in_maps = _shard_inputs(x, w_qkv, b_qkv, w_proj)
    results, _ = run(in_maps)
    return _gather(results, b_proj)


def kernel_timed(x, w_qkv, b_qkv, w_proj, b_proj, iters=5):
    """Like kernel() but also returns per-call device wall times (seconds)."""
    run = _get_runner()
    in_maps = _shard_inputs(x, w_qkv, b_qkv, w_proj)
    results, times = run(in_maps, timing_iters=iters)
    return _gather(results, b_proj), times


def device_time_ns(inputs, loop_n=129, iters=20, rounds=5):
    """Device execution time per kernel invocation (ns), via hardware-loop
    delta: wall(loop_n=N) - wall(loop_n=1) = (N-1) * T_device.  Cancels the
    host/RPC dispatch overhead (~70-140 ms through the axon tunnel), which
    dominates single-call wall time.  Each round pairs a loop_n=1 and a
    loop_n=N measurement under the same network conditions; the median of
    per-round deltas rejects outlier rounds."""
    in_maps = _shard_inputs(inputs["x"], inputs["w_qkv"], inputs["b_qkv"],
                            inputs["w_proj"])
    r1 = _get_runner(loop_n=0)
    rN = _get_runner(loop_n=loop_n)
    deltas = []
    for _ in range(rounds):
        _, t1 = r1(in_maps, timing_iters=iters)
        _, tN = rN(in_maps, timing_iters=iters)
        deltas.append((min(tN) - min(t1)) / (loop_n - 1) * 1e9)
    deltas.sort()
    # lower-median: drift only ever inflates a round, never deflates it
    return deltas[(len(deltas) - 1) // 2]



# revision 46
# speedup vs baseline: 1.1928x; 1.0094x over previous
"""Multi-head attention (B=4, S=2048, E=1024, H=16, Dh=64) on 8 TRN2 NeuronCores.

Sharding: data-parallel over batch (4) x tensor-parallel over head-groups (2).
Core (b, g) computes heads g*8 .. g*8+7 of batch b end-to-end: qkv projection,
attention, and the output-projection partial sum over its 512 attention-output
features.  The host sums the two per-batch partials and adds b_proj.

v4 (vs the fp32r baseline):
  - all matmul operands bf16 (fp32 PSUM accumulate); inputs cast on device
    via gpsimd casting DMAs.  exp writes bf16.  ACT runs exp only.
  - x loaded once (resident bf16 [8][128, 2048]); all inputs arrive as 20
    full-tile casting DMAs per iteration (q|k|v weight slices merged into one
    [128,1536] tile per e-tile), prefetched at the previous iteration's tail.
  - phase A (k/v projection) quarter-interleaved into the first head-pair's
    attention k-loop, so the scalar engine starts exp ~25us into the kernel
    instead of ~90us.
  - q-proj / out-proj PE work emitted as small pieces inside the attention
    k-loop (instead of ahead of each head-pair) to keep ACT fed.
  - attention-output psum is drained to SBUF with one DVE copy per head so
    the psum bank frees ~2us earlier; normalization runs from SBUF.

v8 session notes (all numbers hw-measured via loop-delta microbenches):
  - the kernel is ACT-bound: exp [128,1024] psum->sbuf costs 1403ns back to
    back (785ns fixed overhead + 0.604ns/elem), so 256 exps = 359us of the
    ~405us kernel.  bf16 matmuls N=512 stream at 110ns (2 cols/cycle), so
    the PE (1632 MMs) is only ~180us busy -- NOT the bottleneck.
  - exp free-dim is capped at 1024 by PSUM: scp 2x[128,1024] (4 banks) +
    ops 2x[65,512] (2) + fill 2x[128,512] (2) = all 8 banks.
  - things measured SLOWER and reverted: Schraudolph-on-DVE exp offload of
    4/16 kt tiles (444us plain / 420us with high_priority+retuned kts; the
    cross-engine coupling on the sc double-buffer eats the ACT savings);
    interleaving the rep tail into qc=3 (422us); fp8 PV fails precision
    (3.7e-2 > 2e-2: softmax concentrates on large p, errors don't average).
  - SCORE_LOOKAHEAD (scores one kt ahead of pv) is neutral (405us) -- the
    Tile scheduler already reorders around the exp->pv stall.
"""

import numpy as np
from contextlib import ExitStack

import concourse.bacc as bacc
import concourse.bass as bass
import concourse.tile as tile
import concourse.mybir as mybir

B, S, E, H, DH = 4, 2048, 1024, 16, 64
N_CORES = 8
FG = 512          # features per head-group (8 heads x 64)
HG = 8            # heads per core
ET = E // 128     # 8 e-tiles (qkv contraction)
ST = S // 128     # 16 s-tiles
F32 = mybir.dt.float32
F32R = mybir.dt.float32r
BF16 = mybir.dt.bfloat16

_CACHE: dict = {}
INTERLEAVE_KV = True
QPROJ_HALVES = True
ROTATE_DMA = True
# v7: software-pipeline the rep boundary INTO the last head-pair's attention
# sweep: next rep's kv-quarter-0 + q-proj(qc0) run as qc=3/hp=3 fillers.
# Mode 1: the w/x casting DMAs also move to qc=3's head so the fillers read
#   fresh data.  (measured: -17us SLOWER on hw -- the 20 casting DMAs
#   contend on the Pool queue during qc=3 and the wpT WAR serializes)
# Mode 2: fillers read the CURRENT resident w/x tiles (a WAR against the
#   tail DMAs, which never stalls the readers).  The weights are constant
#   and x is rep-invariant in the timing loop, so the values are identical.
#   Measured 443us with tail DMAs (next rep's kv-quarter-1 stalls on the
#   20-DMA Pool chain) and 444us with pre-epilogue x-first DMAs -- the
#   residual regression is the 13-filler concentration in hp=3 (DVE drain
#   backlog delays the normalization -> epilogue chain).  Every tail-
#   interleave variant measured SLOWER than the plain 30us serial tail;
#   keep False.
TAIL_INTERLEAVE = False
# v8: emit the scores matmuls one kt ahead of the pv matmuls.  The PE queue
# is strict FIFO: with program order [sc(kt), pv(kt), sc(kt+1), ...] the pv
# stalls the queue until exp(kt) completes, which delays sc(kt+1) and hence
# exp(kt+1) -- the ACT engine (the bottleneck at 1403ns per exp) sees a
# bubble every iteration.  Order [sc(kt+1), pv(kt), ...] keeps the next
# exp's input ready before the stall.  (measured neutral on hw)
SCORE_LOOKAHEAD = True
# v11: 1.5-buffered exp -- alternate a 4-bank pair tile A (fd=2048) with a
# 2-bank single tile B (fd=1024): 5 pair-exps + 6 single-exps per loop =
# 18.5us of ACT vs 16x1403=22.4us, and every refill hides under the OTHER
# buffer's exp (unlike v9's single buffer).  Bank budget A4+B2+opp1+fill1=8
# requires: PV split into sequential per-head sweeps (pt tiles buffered in
# SBUF, one [65,512] accumulator bank live at a time) and fill bufs=1.
# MEASURED 476811ns vs the 409585 checkpoint (numerics exact) -- like v9,
# the hidden exp->refill->exp turnaround cost (~1-2us per buffer handoff)
# swamps the 63us ACT-overhead saving.  Keep OFF.
EXP15 = False
# v12: pre-cast x / w_qkv / w_proj to bf16 on the HOST (the kernel casts
# them to bf16 anyway; _shard_inputs already does host-side transposes).
# Halves the per-rep DMA read traffic (16.6MB fp32 -> 8.3MB bf16), which
# shortens the serial DMA chain exposed at the rep boundary.
HOST_BF16 = True
# v13: with bf16 inputs the DMAs no longer cast, so they are no longer
# restricted to the gpsimd queue.  Route the x loads over the sync (HWDGE)
# queue so the two ~11us chains run in parallel, halving the serial DMA
# wall at the rep tail (the tail kv/q-proj compute waits on all of them).
DMA_SPLIT = True
# v14: emit the w/x loads BEFORE the epilogue (their last readers -- the
# qc=2 q-proj fillers -- are long done) so the chains run under the
# epilogue's compute and ahead of its 4 output DMAs on the sync queue;
# only the wpT loads stay behind the epilogue (it reads wpT_t).
DMA_EARLY = True
# v9: halve the ACT instruction count by exping kt-PAIRS: one [128,2048]
# activation (785ns overhead + 0.604ns/elem ~= 2022ns) instead of two
# [128,1024] ones (2x1403ns).  The pair's scores psum tile is 4 banks --
# with opp(2) + fill(2) that is all 8, so the pair buffer is SINGLE-
# buffered: scores(pair p+1) wait for exp(p).  Predicted ~342us; MEASURED
# 497us on hw -- the exposed refill serialization costs ~1.9us/pair, far
# more than the sem+4-matmul model.  Keep OFF.
EXP_PAIRS = False


def _build(debug=False, repeats=1, loop_n=0):
    nc = bacc.Bacc("TRN2", target_bir_lowering=False, debug=False,
                   num_devices=N_CORES)
    IN_DT = BF16 if HOST_BF16 else F32
    xT = nc.dram_tensor("xT", [E, S], IN_DT, kind="ExternalInput").ap()
    wT = nc.dram_tensor("wT", [E, 3 * FG], IN_DT, kind="ExternalInput").ap()
    bqk = nc.dram_tensor("bqk", [2 * FG, 1], F32, kind="ExternalInput").ap()
    bv = nc.dram_tensor("bv", [1, FG], F32, kind="ExternalInput").ap()
    wpT = nc.dram_tensor("wpT", [FG, E], IN_DT, kind="ExternalInput").ap()
    part = nc.dram_tensor("part", [S, E], F32, kind="ExternalOutput").ap()

    Exp = mybir.ActivationFunctionType.Exp

    with tile.TileContext(nc) as tc, ExitStack() as ctx:
        # ---- long-lived tiles ----
        pers = ctx.enter_context(tc.tile_pool(name="pers", bufs=1))

        vb = pers.tile([128, FG], F32, tag="vb")
        nc.sync.dma_start(out=vb, in_=bv.partition_broadcast(128))
        bqk_t = []
        for ft in range(8):
            t = pers.tile([128, 1], F32, tag=f"bqk{ft}", name=f"bqk{ft}")
            nc.sync.dma_start(out=t, in_=bqk[ft * 128:(ft + 1) * 128, :])
            bqk_t.append(t)
        # q/k feature-major tiles: ft 0..3 = q features, 4..7 = k features
        qkT = [pers.tile([128, S], BF16, tag=f"qk{ft}", name=f"qk{ft}")
               for ft in range(8)]
        # v_ext: [s-tile, (8 heads x (64 v cols + ones col))]
        ones = pers.tile([128, HG], F32, tag="ones")
        nc.vector.memset(ones, 1.0)
        vx = []
        for st in range(ST):
            t = pers.tile([128, HG * (DH + 1)], BF16, tag=f"vx{st}",
                          name=f"vx{st}")
            nc.vector.tensor_copy(
                t.rearrange("p (h c) -> p h c", c=DH + 1)[:, :, DH], ones)
            vx.append(t)
        # weights resident in bf16 (cast during DMA on the gpsimd queue);
        # q|k|v slices merged into one tile per e-tile => one DMA each
        wqkv_t = [pers.tile([128, 3 * FG], BF16, tag=f"wqkv{e}",
                            name=f"wqkv{e}") for e in range(ET)]
        wpT_t = [pers.tile([128, E], BF16, tag=f"wp{et}", name=f"wp{et}")
                 for et in range(4)]
        # x resident in bf16, reloaded once per rep
        xb = [pers.tile([128, S], BF16, tag=f"xb{e}", name=f"xb{e}")
              for e in range(ET)]


        fill = ctx.enter_context(tc.tile_pool(name="fill",
                                              bufs=1 if EXP15 else 2,
                                              space="PSUM"))
        # EXP_PAIRS: one [128,2048] pair tile = 4 banks; with opp(2)+fill(2)
        # that is all 8 PSUM banks, so the pair buffer cannot double-buffer
        if EXP15:
            scpA = ctx.enter_context(tc.tile_pool(name="scpA", bufs=1,
                                                  space="PSUM"))
            scpB = ctx.enter_context(tc.tile_pool(name="scpB", bufs=1,
                                                  space="PSUM"))
            scp = None
        else:
            scp = ctx.enter_context(tc.tile_pool(name="scp",
                                                 bufs=1 if EXP_PAIRS else 2,
                                                 space="PSUM"))
        opp = ctx.enter_context(tc.tile_pool(name="opp",
                                             bufs=1 if EXP15 else 2,
                                             space="PSUM"))
        ptp = ctx.enter_context(tc.tile_pool(name="ptp", bufs=4))
        aocp = ctx.enter_context(tc.tile_pool(name="aocp", bufs=2))
        drnp = ctx.enter_context(tc.tile_pool(name="drnp", bufs=3))
        recp = ctx.enter_context(tc.tile_pool(name="recp", bufs=2))
        outp = ctx.enter_context(tc.tile_pool(name="outp", bufs=2))
        rbp = ctx.enter_context(tc.tile_pool(name="rbp", bufs=2))

        # ---------- emission helpers ----------
        # fp32 inputs MUST load via gpsimd (casting DMA); bf16 inputs may
        # split across queues for parallel chains
        x_eng = nc.sync if (HOST_BF16 and DMA_SPLIT) else nc.gpsimd

        def inputs_dma_wx(x_first=False):
            order = ["x", "w"] if x_first else ["w", "x"]
            for which in order:
                for e in range(ET):
                    if which == "w":
                        nc.gpsimd.dma_start(
                            out=wqkv_t[e], in_=wT[e * 128:(e + 1) * 128, :])
                    else:
                        x_eng.dma_start(
                            out=xb[e], in_=xT[e * 128:(e + 1) * 128, :])

        def inputs_dma_wp():
            for et in range(4):
                nc.gpsimd.dma_start(
                    out=wpT_t[et], in_=wpT[et * 128:(et + 1) * 128, :])

        def inputs_dma():
            inputs_dma_wx()
            inputs_dma_wp()

        def kf_piece(sq, kf):
            s0 = sq * 512
            ft = 4 + kf
            pp = fill.tile([128, 512], F32, tag="fl",
                           name=f"kp{sq}_{kf}")
            for e in range(ET):
                nc.tensor.matmul(
                    pp, lhsT=wqkv_t[e][:, FG + kf * 128:FG + (kf + 1) * 128],
                    rhs=xb[e][:, s0:s0 + 512], start=(e == 0),
                    stop=(e == ET - 1))
            nc.vector.tensor_scalar_add(
                out=qkT[ft][:, s0:s0 + 512], in0=pp,
                scalar1=bqk_t[ft])

        def v_piece(st):
            sq, sl = st // 4, st % 4
            s0 = sq * 512
            pp = fill.tile([128, FG], F32, tag="fl",
                           name=f"vp{st}")
            for e in range(ET):
                nc.tensor.matmul(
                    pp, lhsT=xb[e][:, s0 + sl * 128:s0 + (sl + 1) * 128],
                    rhs=wqkv_t[e][:, 2 * FG:3 * FG],
                    start=(e == 0), stop=(e == ET - 1))
            nc.vector.tensor_add(
                out=vx[st].rearrange("p (h c) -> p h c",
                                     c=DH + 1)[:, :, 0:DH],
                in0=pp.rearrange("p (h c) -> p h c", c=DH),
                in1=vb.rearrange("p (h c) -> p h c", c=DH))

        def kv_quarter(sq):
            for kf in range(4):
                kf_piece(sq, kf)
            for sl in range(4):
                v_piece(sq * 4 + sl)

        def q_proj_half(qc, ft, half):
            """4 of the 8 contraction matmuls for one q feature tile."""
            s0 = qc * 512
            pp = fill.tile([128, 512], F32, tag="fl",
                           name=f"qp{qc}_{ft}_{half}")
            es = range(0, 4) if half == 0 else range(4, ET)
            for e in es:
                nc.tensor.matmul(
                    pp, lhsT=wqkv_t[e][:, ft * 128:(ft + 1) * 128],
                    rhs=xb[e][:, s0:s0 + 512],
                    start=(e == es.start), stop=False)
            if half == 0:
                return pp
            nc.vector.tensor_scalar_add(
                out=qkT[ft][:, s0:s0 + 512], in0=pp, scalar1=bqk_t[ft])
            return None

        def q_proj_full(qc, ft):
            s0 = qc * 512
            pp = fill.tile([128, 512], F32, tag="fl",
                           name=f"qpf{qc}_{ft}")
            for e in range(ET):
                nc.tensor.matmul(
                    pp, lhsT=wqkv_t[e][:, ft * 128:(ft + 1) * 128],
                    rhs=xb[e][:, s0:s0 + 512],
                    start=(e == 0), stop=(e == ET - 1))
            nc.vector.tensor_scalar_add(
                out=qkT[ft][:, s0:s0 + 512], in0=pp, scalar1=bqk_t[ft])

        def out_proj(qc_prev, sl, aoc_prev):
            st = qc_prev * 4 + sl
            c0 = st * 128
            lo = sl * 128
            ot = outp.tile([128, E], F32, tag="ot")
            for fc in range(2):
                f0 = fc * 512
                pp = fill.tile([128, 512], F32, tag="fl",
                               name=f"pj{st}_{fc}")
                for et in range(4):
                    nc.tensor.matmul(
                        pp, lhsT=aoc_prev[et][:, lo:lo + 128],
                        rhs=wpT_t[et][:, f0:f0 + 512],
                        start=(et == 0), stop=(et == 3))
                nc.vector.tensor_copy(ot[:, f0:f0 + 512], pp)
            nc.sync.dma_start(out=part[c0:c0 + 128, :], in_=ot)

        # prologue: first iteration's inputs + its k-quarter-0 and qc0
        # q-projection, emitted once ahead of the loop (each iteration's
        # tail then produces these for the next iteration)
        inputs_dma()
        kv_quarter(0)
        for ft in range(4):
            q_proj_full(0, ft)

        import contextlib
        rep_ctx = (tc.For_i(0, loop_n, 1, name="bench")
                   if loop_n else contextlib.nullcontext())
        with rep_ctx:
          for _rep in range(repeats):

            # ---------- emission ----------
            if not INTERLEAVE_KV:
                for sq in range(1, 4):
                    kv_quarter(sq)

            aoc_prev = None
            for qc in range(4):
                q0 = qc * 512
                if qc == 3 and TAIL_INTERLEAVE == 1:
                    # next rep's w_qkv/x casting DMAs: started here so the
                    # tail compute (interleaved into hp=3 below) has its
                    # inputs ~20us before it needs them.  wpT DMAs stay at
                    # the rep tail: the qc=3 out-projection still reads wpT_t.
                    inputs_dma_wx()
                aoc_cur = [aocp.tile([128, 512], BF16, tag=f"aoc{et}",
                                     name=f"aoc{qc}_{et}")
                           for et in range(4)]
                for hp in range(4):
                    # PE filler pieces to emit inside the kt loop, keyed by kt
                    filler = {}
                    if qc == 0 and hp == 0:
                        # overlap remaining k/v quarters with the first
                        # head-pair's attention sweep.  Under EXP15 the
                        # group loop emits scores one GROUP ahead, so each
                        # kv quarter must precede the scores of its first
                        # kt by two groups: kt 1/5/9 (vs 3/7/11).
                        if INTERLEAVE_KV:
                            ks = (1, 5, 9) if EXP15 else (3, 7, 11)
                            filler[ks[0]] = lambda: kv_quarter(1)
                            filler[ks[1]] = lambda: kv_quarter(2)
                            filler[ks[2]] = lambda: kv_quarter(3)
                    elif qc == 3 and hp == 3 and TAIL_INTERLEAVE:
                        # software-pipelined rep boundary: next rep's
                        # kv-quarter-0 and q-projection(qc0) interleave into
                        # the last head-pair's attention sweep (slots chosen
                        # WAR-safe: kf3 after this hp's scores pass kt=3,
                        # v(st) after PV(kt=st))
                        filler[1] = (kf_piece, (0, 0))
                        filler[2] = (out_proj, (2, 3, aoc_prev))
                        filler[3] = (kf_piece, (0, 1))
                        filler[4] = (kf_piece, (0, 2))
                        filler[5] = (kf_piece, (0, 3))
                        filler[6] = (v_piece, (0,))
                        filler[7] = (q_proj_full, (0, 0))
                        filler[8] = (v_piece, (1,))
                        filler[9] = (q_proj_full, (0, 1))
                        filler[10] = (v_piece, (2,))
                        filler[11] = (q_proj_full, (0, 2))
                        filler[12] = (v_piece, (3,))
                        filler[13] = (q_proj_full, (0, 3))
                    else:
                        pieces = []
                        if qc < 3:
                            if qc == 0:
                                # 4 q feature tiles over head-pairs 1..3
                                fts = {1: [0], 2: [1], 3: [2, 3]}[hp]
                                for ft in fts:
                                    if QPROJ_HALVES:
                                        pieces.append(
                                            (q_proj_half, (qc + 1, ft, 0)))
                                        pieces.append(
                                            (q_proj_half, (qc + 1, ft, 1)))
                                    else:
                                        pieces.append(
                                            (q_proj_full, (qc + 1, ft)))
                            elif QPROJ_HALVES:
                                pieces.append((q_proj_half, (qc + 1, hp, 0)))
                                pieces.append((q_proj_half, (qc + 1, hp, 1)))
                            else:
                                pieces.append((q_proj_full, (qc + 1, hp)))
                        if qc > 0:
                            pieces.append((out_proj, (qc - 1, hp, aoc_prev)))
                        slots = [2, 5, 8, 11][:len(pieces)]
                        for s, p in zip(slots, pieces):
                            filler[s] = (p[0], p[1])

                    qTt, kTt = qkT[hp], qkT[4 + hp]
                    if EXP15:
                        # sequential per-head accumulators (1 bank live)
                        ops = [None, None]
                        ops[0] = opp.tile([DH + 1, 512], F32, tag="op",
                                          name=f"op{hp}_{qc}_0")
                    else:
                        ops = []
                        for hh in range(2):
                            op = opp.tile([DH + 1, 512], F32, tag="op",
                                          name=f"op{hp}_{qc}_{hh}")
                            ops.append(op)
                    half_pp = None

                    def emit_filler(kt):
                        nonlocal half_pp
                        f = filler.pop(kt, None)
                        if f is None:
                            return
                        if callable(f):
                            f()
                            return
                        fn, args = f
                        if fn is q_proj_half:
                            if args[2] == 0:
                                half_pp = fn(*args)
                            else:
                                # second half continues on half_pp
                                qc_, ft_, _ = args
                                s0_ = qc_ * 512
                                for e in range(4, ET):
                                    nc.tensor.matmul(
                                        half_pp,
                                        lhsT=wqkv_t[e][:, ft_ * 128:
                                                       (ft_ + 1) * 128],
                                        rhs=xb[e][:, s0_:s0_ + 512],
                                        start=False,
                                        stop=(e == ET - 1))
                                nc.vector.tensor_scalar_add(
                                    out=qkT[ft_][:, s0_:s0_ + 512],
                                    in0=half_pp,
                                    scalar1=bqk_t[ft_])
                                half_pp = None
                        else:
                            fn(*args)

                    def score_mms(sc, col0, kt):
                        k0 = kt * 128
                        for hh in range(2):
                            r = slice(hh * DH, (hh + 1) * DH)
                            nc.tensor.matmul(
                                sc[:, col0 + hh * 512:col0 + (hh + 1) * 512],
                                lhsT=kTt[r, k0:k0 + 128],
                                rhs=qTt[r, q0:q0 + 512],
                                start=True, stop=True)

                    def pv_mms(pt, col0, kt):
                        for hh in range(2):
                            h = hp * 2 + hh
                            nc.tensor.matmul(
                                ops[hh],
                                lhsT=vx[kt][:, h * (DH + 1):
                                            (h + 1) * (DH + 1)],
                                rhs=pt[:, col0 + hh * 512:
                                        col0 + (hh + 1) * 512],
                                start=(kt == 0), stop=(kt == ST - 1))

                    if EXP15:
                        # 1.5-buffered exp: groups alternate the 4-bank A
                        # pair tile (fd=2048) and the 2-bank B tile
                        # (fd=1024); the next group's scores are emitted
                        # right after this group's exp, so every refill
                        # hides under the other buffer's exp.  PV hh=0 runs
                        # in-loop; hh=1 sweeps afterwards from the buffered
                        # pt tiles (ops bank freed by then).
                        GROUPS = [("A", (0, 1)), ("B", (2,)),
                                  ("A", (3, 4)), ("B", (5,)),
                                  ("A", (6, 7)), ("B", (8,)),
                                  ("A", (9, 10)), ("B", (11,)),
                                  ("A", (12, 13)), ("B", (14,)),
                                  ("B", (15,))]

                        def pv_one(op, hh, pt, col0, kt):
                            h = hp * 2 + hh
                            nc.tensor.matmul(
                                op,
                                lhsT=vx[kt][:, h * (DH + 1):
                                            (h + 1) * (DH + 1)],
                                rhs=pt[:, col0 + hh * 512:
                                        col0 + (hh + 1) * 512],
                                start=(kt == 0), stop=(kt == ST - 1))

                        def group_scores(g):
                            pool_, kts_ = GROUPS[g]
                            p = scpA if pool_ == "A" else scpB
                            sc = p.tile([128, 1024 * len(kts_)], F32,
                                        tag="sc", name=f"sc{hp}_{qc}_g{g}")
                            for j, kt in enumerate(kts_):
                                score_mms(sc, j * 1024, kt)
                            return sc

                        pts = []
                        sc_cur = group_scores(0)
                        for g, (pool_, kts_) in enumerate(GROUPS):
                            pt = ptp.tile([128, 1024 * len(kts_)], BF16,
                                          tag=f"pt{pool_}",
                                          bufs=6 if pool_ == "A" else 8,
                                          name=f"pt{hp}_{qc}_g{g}")
                            nc.scalar.activation(pt, sc_cur, Exp,
                                                 scale=0.125)
                            if g + 1 < len(GROUPS):
                                sc_cur = group_scores(g + 1)
                            for j, kt in enumerate(kts_):
                                pts.append((pt, j * 1024, kt))
                                pv_one(ops[0], 0, pt, j * 1024, kt)
                                emit_filler(kt)
                        # hh=1 sweep from the buffered pt tiles; its PE work
                        # hides under the next loop's exps
                        ops[1] = opp.tile([DH + 1, 512], F32, tag="op",
                                          name=f"op{hp}_{qc}_1")
                        for (pt, c0, kt) in pts:
                            pv_one(ops[1], 1, pt, c0, kt)
                    elif EXP_PAIRS:
                        # one [128,2048] exp per kt-PAIR; the 4-bank pair
                        # psum is single-buffered (all 8 banks committed).
                        # PE-queue order per pair (FIFO!): fillers run while
                        # the ACT exps, the NEXT pair's scores go before the
                        # pv so the ACT restarts after only ~4 matmuls, and
                        # the pv (which only needs pt in SBUF) trails.
                        def pair_scores(kp):
                            sc = scp.tile([128, 2048], F32, tag="sc",
                                          name=f"sc{hp}_{qc}_{kp}")
                            for j in range(2):
                                score_mms(sc, j * 1024, 2 * kp + j)
                            return sc

                        sc_cur = pair_scores(0)
                        for kp in range(ST // 2):
                            pt = ptp.tile([128, 2048], BF16, tag="pt",
                                          name=f"pt{hp}_{qc}_{kp}")
                            nc.scalar.activation(pt, sc_cur, Exp, scale=0.125)
                            emit_filler(2 * kp)
                            emit_filler(2 * kp + 1)
                            if kp + 1 < ST // 2:
                                sc_cur = pair_scores(kp + 1)
                            for j in range(2):
                                pv_mms(pt, j * 1024, 2 * kp + j)
                    else:
                        # lookahead is ILLEGAL in the (qc0, hp0) loop: its
                        # fillers (kv quarters 1-3) produce the k-features
                        # that scores(kt+1) consumes -- hoisting the scores
                        # above the filler would read stale qkT
                        la = SCORE_LOOKAHEAD and not (
                            qc == 0 and hp == 0 and INTERLEAVE_KV)
                        sc_next = None
                        for kt in range(ST):
                            if sc_next is None:
                                sc_next = scp.tile([128, 1024], F32,
                                                   tag="sc",
                                                   name=f"sc{hp}_{qc}_{kt}")
                                score_mms(sc_next, 0, kt)
                            sc = sc_next
                            sc_next = None
                            if la and kt + 1 < ST:
                                sc_next = scp.tile([128, 1024], F32,
                                                   tag="sc",
                                                   name=f"sc{hp}_{qc}_{kt+1}")
                                score_mms(sc_next, 0, kt + 1)
                            pt = ptp.tile([128, 1024], BF16, tag="pt",
                                          name=f"pt{hp}_{qc}_{kt}")
                            nc.scalar.activation(pt, sc, Exp, scale=0.125)
                            pv_mms(pt, 0, kt)
                            emit_filler(kt)
                    # any unemitted filler (shouldn't happen, but be safe)
                    for kt in sorted(filler):
                        f = filler[kt]
                        if callable(f):
                            f()
                        else:
                            fn, args = f
                            fn(*args)
                    for hh in range(2):
                        # one DVE copy drains the psum accumulator (freeing
                        # the bank); normalization runs from SBUF
                        drn = drnp.tile([DH + 1, 512], F32, tag="drn",
                                        name=f"drn{hp}_{qc}_{hh}")
                        nc.vector.tensor_copy(drn, ops[hh])
                        srow = recp.tile([1, 512], F32, tag="srow")
                        nc.vector.tensor_copy(srow, drn[DH:DH + 1, :])
                        rec = recp.tile([1, 512], F32, tag="rec")
                        nc.vector.reciprocal_approx_fast(rec, srow)
                        rb = rbp.tile([DH, 512], F32, tag="rb")
                        nc.gpsimd.partition_broadcast(rb, rec)
                        nc.vector.tensor_mul(
                            out=aoc_cur[hp][hh * DH:(hh + 1) * DH, :],
                            in0=drn[0:DH, :], in1=rb)
                aoc_prev = aoc_cur
            if TAIL_INTERLEAVE == 2 or (DMA_EARLY and not TAIL_INTERLEAVE):
                # x/w DMAs AFTER the qc=3 loops but BEFORE the epilogue, x
                # first: both queue chains run under the epilogue compute
                # and ahead of its output DMAs on the sync queue, so the
                # tail kv/q-proj compute waits far less.
                inputs_dma_wx(x_first=True)
            # epilogue: out projection for the last chunk.  With EXP_PAIRS the
            # scp pair tile is single-buffered, so run the epilogue through
            # the fill pool (2 bufs) to keep two tiles in flight; otherwise
            # use the (now idle) scp banks as before.
            if EXP_PAIRS or EXP15:
                for sl in range(4):
                    st = 3 * 4 + sl
                    c0 = st * 128
                    lo = sl * 128
                    ot = outp.tile([128, E], F32, tag="ot")
                    for fc in range(2):
                        f0 = fc * 512
                        pp = fill.tile([128, 512], F32, tag="fl",
                                       name=f"ep{sl}_{fc}")
                        for et in range(4):
                            nc.tensor.matmul(
                                pp,
                                lhsT=aoc_prev[et][:, lo:lo + 128],
                                rhs=wpT_t[et][:, f0:f0 + 512],
                                start=(et == 0), stop=(et == 3))
                        nc.vector.tensor_copy(ot[:, f0:f0 + 512], pp)
                    nc.sync.dma_start(out=part[c0:c0 + 128, :], in_=ot)
            else:
                for sl in range(4):
                    st = 3 * 4 + sl
                    c0 = st * 128
                    lo = sl * 128
                    pp = scp.tile([128, 1024], F32, tag="sc", name=f"ep{sl}")
                    for fc in range(2):
                        f0 = fc * 512
                        for et in range(4):
                            nc.tensor.matmul(
                                pp[:, f0:f0 + 512],
                                lhsT=aoc_prev[et][:, lo:lo + 128],
                                rhs=wpT_t[et][:, f0:f0 + 512],
                                start=(et == 0), stop=(et == 3))
                    ot = outp.tile([128, E], F32, tag="ot")
                    nc.vector.tensor_copy(ot, pp)
                    nc.sync.dma_start(out=part[c0:c0 + 128, :], in_=ot)

            # prefetch the next iteration's inputs and precompute its
            # k-quarter-0 / q-projection(qc0) so the next iteration opens
            # directly with score matmuls (software-pipelined loop boundary)
            if TAIL_INTERLEAVE == 1:
                # kv-quarter-0 / q-proj(0) already ran as hp=3 fillers;
                # only the out-projection weights remain (WAR vs epilogue)
                inputs_dma_wp()
            elif TAIL_INTERLEAVE == 2:
                # x/w DMAs already issued before the epilogue; only the
                # out-projection weights remain (WAR vs the epilogue reads)
                inputs_dma_wp()
            else:
                if DMA_EARLY:
                    inputs_dma_wp()     # w/x already issued pre-epilogue
                else:
                    inputs_dma()
                kv_quarter(0)
                for ft in range(4):
                    q_proj_full(0, ft)

    nc.compile()
    return nc


def _get_runner(debug=False, repeats=1, loop_n=0):
    """Build (once) a cached jit'd SPMD runner over the 8 axon cores."""
    key = ("run", debug, repeats, loop_n)
    if key in _CACHE:
        return _CACHE[key]

    import jax
    from jax.experimental.shard_map import shard_map
    from jax.sharding import Mesh, PartitionSpec, NamedSharding
    from concourse.bass2jax import (install_neuronx_cc_hook, _bass_exec_p,
                                    partition_id_tensor)

    nc = _build(debug, repeats, loop_n)
    install_neuronx_cc_hook()

    in_names, out_names, out_avals, zero_outs = [], [], [], []
    partition_name = nc.partition_id_tensor.name if nc.partition_id_tensor else None
    for alloc in nc.m.functions[0].allocations:
        if not isinstance(alloc, mybir.MemoryLocationSet):
            continue
        name = alloc.memorylocations[0].name
        if alloc.kind == "ExternalInput":
            if name != partition_name:
                in_names.append(name)
        elif alloc.kind == "ExternalOutput":
            shape = tuple(alloc.tensor_shape)
            dtype = mybir.dt.np(alloc.dtype)
            out_names.append(name)
            out_avals.append(jax.core.ShapedArray(shape, dtype))
            zero_outs.append(np.zeros(shape, dtype))
    n_params = len(in_names)
    n_outs = len(out_names)
    all_in_names = in_names + out_names
    if partition_name is not None:
        all_in_names.append(partition_name)

    def _body(*args):
        operands = list(args)
        if partition_name is not None:
            operands.append(partition_id_tensor())
        outs = _bass_exec_p.bind(
            *operands,
            out_avals=tuple(out_avals),
            in_names=tuple(all_in_names),
            out_names=tuple(out_names),
            lowering_input_output_aliases=(),
            sim_require_finite=True,
            sim_require_nnan=True,
            nc=nc,
        )
        return tuple(outs)

    devices = jax.devices()[:N_CORES]
    mesh = Mesh(np.asarray(devices), ("core",))
    in_specs = (PartitionSpec("core"),) * (n_params + n_outs)
    out_specs = (PartitionSpec("core"),) * n_outs
    sharded = jax.jit(
        shard_map(_body, mesh=mesh, in_specs=in_specs, out_specs=out_specs,
                  check_rep=False),
        donate_argnums=tuple(range(n_params, n_params + n_outs)),
        keep_unused=True,
    )
    sharded_nodonate = jax.jit(
        shard_map(_body, mesh=mesh, in_specs=in_specs, out_specs=out_specs,
                  check_rep=False),
        keep_unused=True,
    )
    core_sharding = NamedSharding(mesh, PartitionSpec("core"))

    def run(in_maps, timing_iters=0):
        concat_in = [
            np.concatenate([np.asarray(m[name]) for m in in_maps], axis=0)
            for name in in_names
        ]
        concat_zeros = [
            np.zeros((N_CORES * z.shape[0], *z.shape[1:]), z.dtype)
            for z in zero_outs
        ]
        out_arrs = sharded(*concat_in, *concat_zeros)
        results = [
            {name: np.asarray(out_arrs[i]).reshape(N_CORES, *out_avals[i].shape)[c]
             for i, name in enumerate(out_names)}
            for c in range(N_CORES)
        ]
        times = []
        if timing_iters:
            import time
            dev = [jax.device_put(a, core_sharding)
                   for a in concat_in + concat_zeros]
            jax.block_until_ready(dev)
            for _ in range(2):
                jax.block_until_ready(sharded_nodonate(*dev))
            for _ in range(timing_iters):
                t0 = time.perf_counter()
                jax.block_until_ready(sharded_nodonate(*dev))
                times.append(time.perf_counter() - t0)
        return results, times

    _CACHE[key] = run
    return run


def _shard_inputs(x, w_qkv, b_qkv, w_proj):
    x = np.asarray(x, np.float32)
    w = np.asarray(w_qkv, np.float32)
    bq = np.asarray(b_qkv, np.float32)
    wp = np.asarray(w_proj, np.float32)
    if HOST_BF16:
        import ml_dtypes
        in_dt = ml_dtypes.bfloat16
    else:
        in_dt = np.float32
    in_maps = []
    for b in range(B):
        xTb = np.ascontiguousarray(x[b].T.astype(in_dt))        # [E, S]
        for g in range(2):
            r = slice(g * FG, (g + 1) * FG)
            w_slice = np.concatenate([w[0:E][r], w[E:2 * E][r],
                                      w[2 * E:3 * E][r]], axis=0)  # [1536, E]
            in_maps.append({
                "xT": xTb,
                "wT": np.ascontiguousarray(w_slice.T.astype(in_dt)),
                "bqk": np.concatenate([bq[0:E][r], bq[E:2 * E][r]]
                                      ).reshape(2 * FG, 1).astype(np.float32),
                "bv": bq[2 * E:3 * E][r].reshape(1, FG).astype(np.float32),
                "wpT": np.ascontiguousarray(wp[:, r].T.astype(in_dt)),
            })
    return in_maps


def _gather(results, b_proj):
    bp = np.asarray(b_proj, np.float32)
    out = np.empty((B, S, E), np.float32)
    for b in range(B):
        out[b] = results[2 * b]["part"] + results[2 * b + 1]["part"] + bp
    return out


def kernel(x, w_qkv, b_qkv, w_proj, b_proj):
    run = _get_runner()
    in_maps = _shard_inputs(x, w_qkv, b_qkv, w_proj)
    results, _ = run(in_maps)
    return _gather(results, b_proj)


def kernel_timed(x, w_qkv, b_qkv, w_proj, b_proj, iters=5):
    """Like kernel() but also returns per-call device wall times (seconds)."""
    run = _get_runner()
    in_maps = _shard_inputs(x, w_qkv, b_qkv, w_proj)
    results, times = run(in_maps, timing_iters=iters)
    return _gather(results, b_proj), times


def device_time_ns(inputs, loop_n=129, iters=20, rounds=5):
    """Device execution time per kernel invocation (ns), via hardware-loop
    delta: wall(loop_n=N) - wall(loop_n=1) = (N-1) * T_device.  Cancels the
    host/RPC dispatch overhead (~70-140 ms through the axon tunnel), which
    dominates single-call wall time.  Each round pairs a loop_n=1 and a
    loop_n=N measurement under the same network conditions; the median of
    per-round deltas rejects outlier rounds."""
    in_maps = _shard_inputs(inputs["x"], inputs["w_qkv"], inputs["b_qkv"],
                            inputs["w_proj"])
    r1 = _get_runner(loop_n=0)
    rN = _get_runner(loop_n=loop_n)
    deltas = []
    for _ in range(rounds):
        _, t1 = r1(in_maps, timing_iters=iters)
        _, tN = rN(in_maps, timing_iters=iters)
        deltas.append((min(tN) - min(t1)) / (loop_n - 1) * 1e9)
    deltas.sort()
    # lower-median: drift only ever inflates a round, never deflates it
    return deltas[(len(deltas) - 1) // 2]

